# revision 1
# baseline (speedup 1.0000x reference)
"""MoE feed-forward (8 experts, top-2, 2 shared experts) on 8 TRN2 NeuronCores.

Strategy (expert-parallel):
  - 1 expert per core. Router computed on-device per core for its own 1/8
    token slice (token-major), then a tiny AllToAll distributes comb columns
    so core e ends up with gate weights of expert e for ALL tokens.
  - Dense expert FFN per core in fp32r (stage1, feature-major hidden) /
    bf16 (stage2, token-major output). Gate scaling is a per-partition
    tensor_scalar on the token-major output.
  - ReduceScatter sums expert contributions across cores and hands each
    core its own token slice.
  - Shared experts are computed data-parallel (each core: its token slice),
    overlapping the ReduceScatter, and added locally before writing out.
"""

import sys

if "/opt/trn_rl_repo" not in sys.path:
    sys.path.insert(0, "/opt/trn_rl_repo")

import numpy as np
import ml_dtypes

import concourse.bass as bass
import concourse.tile as tile
import concourse.mybir as mybir
from concourse import bacc
from concourse.bass_utils import run_bass_kernel_spmd

F32 = mybir.dt.float32
F32R = mybir.dt.float32r
BF16 = mybir.dt.bfloat16
I32 = mybir.dt.int32
AX = mybir.AxisListType
ALU = mybir.AluOpType
ACTF = mybir.ActivationFunctionType

N, D, HE, E, S = 8192, 1024, 2048, 8, 2
NCORES = 8
NT = N // NCORES      # 1024 tokens per core slice
TBC = 1024            # token chunk for the dense expert stages
NTBC = N // TBC
ND = D // 128         # 8
NH = HE // 128        # 16
RG = [list(range(NCORES))]

CAPQ_HOST = 640       # sparse: per-(expert, quarter) slot capacity

_NC_CACHE = {}


def _build(has_rb, has_b2, has_sb2, debug=False):
    nc = bacc.Bacc(None, target_bir_lowering=False)

    xt_p = nc.declare_dram_parameter("xt", [D, N], F32R, isOutput=False)
    xsl_p = nc.declare_dram_parameter("xsl", [D, NT], F32R, isOutput=False)
    w1_p = nc.declare_dram_parameter("w1", [D, HE], F32R, isOutput=False)
    w2_p = nc.declare_dram_parameter("w2", [HE, D], BF16, isOutput=False)
    rw_p = nc.declare_dram_parameter("rw", [128, ND, E], F32R, isOutput=False)
    sw1_p = nc.declare_dram_parameter("sw1", [S * D, HE], F32R, isOutput=False)
    sw2_p = nc.declare_dram_parameter("sw2", [S * HE, D], BF16, isOutput=False)
    b1v_p = nc.declare_dram_parameter("b1v", [128, NH], F32, isOutput=False)
    sb1v_p = nc.declare_dram_parameter("sb1v", [128, S * NH], F32, isOutput=False)
    id_p = nc.declare_dram_parameter("ident", [128, 128], F32, isOutput=False)
    if has_rb:
        rb_p = nc.declare_dram_parameter("rbr", [128, E], F32, isOutput=False)
    if has_b2:
        b2_p = nc.declare_dram_parameter("b2r", [128, D], F32, isOutput=False)
    if has_sb2:
        sb2_p = nc.declare_dram_parameter("sb2r", [128, D], F32, isOutput=False)
    yo_p = nc.declare_dram_parameter("y_out", [NT, D], F32, isOutput=True)
    if debug:
        dbg_combT_p = nc.declare_dram_parameter("dbg_combT", [E, NT], F32, isOutput=True)
        dbg_ctm_p = nc.declare_dram_parameter("dbg_ctm", [128, N // 128], F32, isOutput=True)
        dbg_ysh_p = nc.declare_dram_parameter("dbg_ysh", [NT, D], F32, isOutput=True)

    from contextlib import ExitStack

    with tile.TileContext(nc) as tc, ExitStack() as ctx:
        ep = ctx.enter_context
        dram = ep(tc.tile_pool(name="dram", bufs=1, space="DRAM"))
        cpool = ep(tc.tile_pool(name="cpool", bufs=1))
        xslp = ep(tc.tile_pool(name="xslp", bufs=1))
        xtp = ep(tc.tile_pool(name="xtp", bufs=1))
        htp = ep(tc.tile_pool(name="htp", bufs=1))
        w2rp = ep(tc.tile_pool(name="w2rp", bufs=1))
        wst = ep(tc.tile_pool(name="wst", bufs=6))
        sw2st = ep(tc.tile_pool(name="sw2st", bufs=3))
        ysbp = ep(tc.tile_pool(name="ysbp", bufs=2))
        finp = ep(tc.tile_pool(name="finp", bufs=2))
        rp = ep(tc.tile_pool(name="rp", bufs=2))
        ps1 = ep(tc.tile_pool(name="ps1", bufs=2, space="PSUM"))
        ps2 = ep(tc.tile_pool(name="ps2", bufs=2, space="PSUM"))

        moe_y = dram.tile([N, D], F32, name="moe_y")
        rs_out = dram.tile([NT, D], F32, name="rs_out")
        a2a_in = dram.tile([E, NT], F32, name="a2a_in")
        a2a_out = dram.tile([E, NT], F32, name="a2a_out")

        ident = cpool.tile([128, 128], F32, name="ident")
        nc.sync.dma_start(ident[:], id_p[:])
        b1v = cpool.tile([128, NH], F32, name="b1v")
        nc.sync.dma_start(b1v[:], b1v_p[:])
        sb1v = cpool.tile([128, S * NH], F32, name="sb1v")
        nc.sync.dma_start(sb1v[:], sb1v_p[:])
        # router path in true fp32 tiles (PE matmul mode follows the SBUF
        # tensor dtype; fp32r noise ~3e-4 exceeds the smallest gate gap 2e-5)
        rw_sb = cpool.tile([128, ND, E], F32, name="rw_sb")
        nc.sync.dma_start(rw_sb[:], rw_p[:].bitcast(F32))
        if has_rb:
            rbr = cpool.tile([128, E], F32, name="rbr")
            nc.sync.dma_start(rbr[:], rb_p[:])
        if has_b2:
            b2r = cpool.tile([128, D], F32, name="b2r")
            nc.sync.dma_start(b2r[:], b2_p[:])
        if has_sb2:
            sb2r = cpool.tile([128, D], F32, name="sb2r")
            nc.sync.dma_start(sb2r[:], sb2_p[:])

        xsl = []
        for d in range(ND):
            t = xslp.tile([128, NT], F32R, tag=f"xsl{d}", name=f"xsl{d}")
            nc.sync.dma_start(t[:], xsl_p[d * 128 : (d + 1) * 128, :])
            xsl.append(t)

        # ---------------- router (own token slice, token-major) ----------
        combT = cpool.tile([E, NT], F32, name="combT")
        for j in range(NT // 128):
            pg = ps1.tile([128, E], F32, tag="hpsum", name=f"pg{j}")
            for d in range(ND):
                xr = rp.tile([128, 128], F32, tag="xr", name=f"xr{j}_{d}", bufs=4)
                nc.sync.dma_start(
                    xr[:],
                    xsl_p[d * 128 : (d + 1) * 128, j * 128 : (j + 1) * 128].bitcast(F32),
                )
                nc.tensor.matmul(
                    pg[:],
                    xr[:],
                    rw_sb[:, d, :],
                    start=(d == 0),
                    stop=(d == ND - 1),
                )
            gates = rp.tile([128, E], F32, tag="gates", name=f"gates{j}")
            if has_rb:
                nc.vector.tensor_tensor(gates[:], pg[:], rbr[:], op=ALU.add)
            else:
                nc.vector.tensor_copy(gates[:], pg[:])
            m1 = rp.tile([128, 1], F32, tag="m1", name=f"m1_{j}")
            nc.vector.tensor_reduce(m1[:], gates[:], axis=AX.X, op=ALU.max)
            mask1 = rp.tile([128, E], F32, tag="mask1", name=f"mask1_{j}")
            nc.vector.tensor_scalar(mask1[:], gates[:], m1[:], None, op0=ALU.is_equal)
            negm = rp.tile([128, E], F32, tag="negm", name=f"negm{j}")
            nc.vector.tensor_scalar(negm[:], mask1[:], -1e30, None, op0=ALU.mult)
            gm = rp.tile([128, E], F32, tag="gm", name=f"gm{j}")
            nc.vector.tensor_tensor(gm[:], gates[:], negm[:], op=ALU.add)
            m2 = rp.tile([128, 1], F32, tag="m2", name=f"m2_{j}")
            nc.vector.tensor_reduce(m2[:], gm[:], axis=AX.X, op=ALU.max)
            mask2 = rp.tile([128, E], F32, tag="mask2", name=f"mask2_{j}")
            nc.vector.tensor_scalar(mask2[:], gm[:], m2[:], None, op0=ALU.is_equal)
            dl = rp.tile([128, 1], F32, tag="dl", name=f"dl{j}")
            nc.vector.tensor_tensor(dl[:], m2[:], m1[:], op=ALU.subtract)
            e2 = rp.tile([128, 1], F32, tag="e2", name=f"e2_{j}")
            nc.scalar.activation(e2[:], dl[:], ACTF.Exp)
            den = rp.tile([128, 1], F32, tag="den", name=f"den{j}")
            nc.vector.tensor_scalar_add(den[:], e2[:], 1.0)
            p1 = rp.tile([128, 1], F32, tag="p1", name=f"p1_{j}")
            nc.vector.reciprocal(p1[:], den[:])
            p2 = rp.tile([128, 1], F32, tag="p2", name=f"p2_{j}")
            nc.vector.tensor_tensor(p2[:], e2[:], p1[:], op=ALU.mult)
            t1 = rp.tile([128, E], F32, tag="t1", name=f"t1_{j}")
            nc.vector.tensor_scalar(t1[:], mask1[:], p1[:], None, op0=ALU.mult)
            t2 = rp.tile([128, E], F32, tag="t2", name=f"t2_{j}")
            nc.vector.tensor_scalar(t2[:], mask2[:], p2[:], None, op0=ALU.mult)
            cj = rp.tile([128, E], F32, tag="cj", name=f"cj{j}")
            nc.vector.tensor_tensor(cj[:], t1[:], t2[:], op=ALU.add)
            pt = ps1.tile([E, 128], F32, tag="hpsum", name=f"pt{j}")
            nc.tensor.transpose(pt[:], cj[:], ident[:])
            nc.vector.tensor_copy(combT[:, j * 128 : (j + 1) * 128], pt[:])
        nc.sync.dma_start(a2a_in[:], combT[:])
        nc.gpsimd.collective_compute(
            "AllToAll",
            ALU.bypass,
            replica_groups=RG,
            ins=[a2a_in.opt()],
            outs=[a2a_out.opt()],
        )
        comb_tm = cpool.tile([128, N // 128], F32, name="comb_tm")
        nc.sync.dma_start(
            comb_tm[:], a2a_out[:].rearrange("a (c p) -> p (a c)", p=128)
        )
        if debug:
            nc.sync.dma_start(dbg_combT_p[:], combT[:])
            nc.sync.dma_start(dbg_ctm_p[:], comb_tm[:])

        # ---------------- dense expert FFN ------------------------------
        w2res = []
        for h in range(NH):
            t = w2rp.tile([128, D], BF16, tag=f"w2r{h}", name=f"w2r{h}")
            nc.sync.dma_start(t[:], w2_p[h * 128 : (h + 1) * 128, :])
            w2res.append(t)

        for tb in range(NTBC):
            xts = []
            for d in range(ND):
                t = xtp.tile([128, TBC], F32R, tag=f"xt{d}", name=f"xt{tb}_{d}")
                nc.sync.dma_start(t[:], xt_p[d * 128 : (d + 1) * 128, tb * TBC : (tb + 1) * TBC])
                xts.append(t)
            hts = []
            for h in range(NH):
                ph = ps1.tile([128, TBC], F32, tag="hpsum", name=f"ph{tb}_{h}")
                for d in range(ND):
                    w1t = wst.tile([128, 128], F32R, tag="w1t", name=f"w1t{tb}_{h}_{d}")
                    nc.sync.dma_start(w1t[:], w1_p[d * 128 : (d + 1) * 128, h * 128 : (h + 1) * 128])
                    for v in range(TBC // 512):
                        nc.tensor.matmul(
                            ph[:, v * 512 : (v + 1) * 512],
                            w1t[:],
                            xts[d][:, v * 512 : (v + 1) * 512],
                            start=(d == 0),
                            stop=(d == ND - 1),
                        )
                ht = htp.tile([128, TBC], BF16, tag=f"hT{h}", name=f"hT{tb}_{h}")
                nc.scalar.activation(ht[:], ph[:], ACTF.Gelu, bias=b1v[:, h : h + 1])
                hts.append(ht)
            for k in range(TBC // 128):
                g = tb * (TBC // 128) + k
                py = ps2.tile([128, D], F32, tag="ypsum", name=f"py{g}")
                for h in range(NH):
                    for v in range(D // 512):
                        nc.tensor.matmul(
                            py[:, v * 512 : (v + 1) * 512],
                            hts[h][:, k * 128 : (k + 1) * 128],
                            w2res[h][:, v * 512 : (v + 1) * 512],
                            start=(h == 0),
                            stop=(h == NH - 1),
                        )
                if has_b2:
                    nc.vector.tensor_tensor(py[:], py[:], b2r[:], op=ALU.add)
                ysb = ysbp.tile([128, D], F32, tag="ysb", name=f"ysb{g}")
                nc.vector.tensor_scalar(ysb[:], py[:], comb_tm[:, g : g + 1], None, op0=ALU.mult)
                nc.sync.dma_start(moe_y[g * 128 : (g + 1) * 128, :], ysb[:])

        # ---------------- combine across cores --------------------------
        nc.gpsimd.collective_compute(
            "ReduceScatter",
            ALU.add,
            replica_groups=RG,
            ins=[moe_y.opt()],
            outs=[rs_out.opt()],
        )

        # ---------------- shared experts (overlap the RS) ----------------
        ysh = []
        for k in range(NT // 128):
            t = xtp.tile([128, D], F32, tag=f"xt{k}", name=f"ysh{k}")
            ysh.append(t)
        for s in range(S):
            shts = []
            for h in range(NH):
                ph = ps1.tile([128, NT], F32, tag="hpsum", name=f"shp{s}_{h}")
                for d in range(ND):
                    swt = wst.tile([128, 128], F32R, tag="w1t", name=f"swt{s}_{h}_{d}")
                    nc.sync.dma_start(swt[:], sw1_p[s * D + d * 128 : s * D + (d + 1) * 128, h * 128 : (h + 1) * 128])
                    for v in range(NT // 512):
                        nc.tensor.matmul(
                            ph[:, v * 512 : (v + 1) * 512],
                            swt[:],
                            xsl[d][:, v * 512 : (v + 1) * 512],
                            start=(d == 0),
                            stop=(d == ND - 1),
                        )
                sht = htp.tile([128, NT], BF16, tag=f"hT{h}", name=f"shT{s}_{h}")
                nc.scalar.activation(sht[:], ph[:], ACTF.Gelu, bias=sb1v[:, s * NH + h : s * NH + h + 1])
                shts.append(sht)
            for kg in range(NT // 256):
                pys = []
                for ki in range(2):
                    k = kg * 2 + ki
                    pys.append(ps2.tile([128, D], F32, tag="ypsum", name=f"spy{s}_{k}"))
                for h in range(NH):
                    sw2t = sw2st.tile([128, D], BF16, tag="sw2t", name=f"sw2t{s}_{kg}_{h}")
                    nc.sync.dma_start(sw2t[:], sw2_p[s * HE + h * 128 : s * HE + (h + 1) * 128, :])
                    for ki in range(2):
                        k = kg * 2 + ki
                        for v in range(D // 512):
                            nc.tensor.matmul(
                                pys[ki][:, v * 512 : (v + 1) * 512],
                                shts[h][:, k * 128 : (k + 1) * 128],
                                sw2t[:, v * 512 : (v + 1) * 512],
                                start=(h == 0),
                                stop=(h == NH - 1),
                            )
                for ki in range(2):
                    k = kg * 2 + ki
                    if s == 0:
                        nc.vector.tensor_copy(ysh[k][:], pys[ki][:])
                    else:
                        nc.vector.tensor_tensor(ysh[k][:], ysh[k][:], pys[ki][:], op=ALU.add)

        # ---------------- final: rs slice + shared ------------------------
        for k in range(NT // 128):
            fin = finp.tile([128, D], F32, tag="fin", name=f"fin{k}")
            if debug:
                nc.sync.dma_start(dbg_ysh_p[k * 128 : (k + 1) * 128, :], ysh[k][:])
            nc.sync.dma_start(fin[:], rs_out[k * 128 : (k + 1) * 128, :])
            nc.vector.tensor_tensor(fin[:], fin[:], ysh[k][:], op=ALU.add)
            if has_sb2:
                nc.vector.tensor_tensor(fin[:], fin[:], sb2r[:], op=ALU.add)
            nc.sync.dma_start(yo_p[k * 128 : (k + 1) * 128, :], fin[:])

    nc.compile()
    return nc


def _build_sparse(has_rb, has_b2, has_sb2):
    """Optimized sparse expert-parallel kernel (v2).

    Queue discipline is the key design constraint: each DMA issue costs
    ~600ns on its HWDGE engine, so bulk loads use full-row DMAs, sw1
    streaming is split across the sync+scalar queues, gathered-token
    transposes run on the PE (drained by vector), and the compaction
    relayout sits on sync between the two shared half-passes.
    """
    from concourse import library_config

    nc = bacc.Bacc(None, target_bir_lowering=False)

    NQ = 4                 # token quarters (chunked ReduceScatter)
    QTOK = N // NQ         # 2048 tokens per quarter
    CAPQ = 640             # per-(expert, quarter) slot capacity (max seen 559)
    CBQ = CAPQ // 128      # 5 slot blocks per quarter
    CFQ = CAPQ // 16       # sparse_gather out free dim

    xtm_p = nc.declare_dram_parameter("xtm", [N, D], BF16, isOutput=False)
    xsl_p = nc.declare_dram_parameter("xsl", [D, NT], F32, isOutput=False)
    xslb_p = nc.declare_dram_parameter("xslb", [D, NT], BF16, isOutput=False)
    w1_p = nc.declare_dram_parameter("w1", [D, HE], BF16, isOutput=False)
    w2_p = nc.declare_dram_parameter("w2", [HE, D], BF16, isOutput=False)
    rw_p = nc.declare_dram_parameter("rw", [128, ND, E], F32, isOutput=False)
    sw1_p = nc.declare_dram_parameter("sw1", [S * D, HE], BF16, isOutput=False)
    sw2_p = nc.declare_dram_parameter("sw2", [S * HE, D], BF16, isOutput=False)
    b1v_p = nc.declare_dram_parameter("b1v", [128, NH], F32, isOutput=False)
    sb1v_p = nc.declare_dram_parameter("sb1v", [128, S * NH], F32, isOutput=False)
    id_p = nc.declare_dram_parameter("ident", [128, 128], F32, isOutput=False)
    io16_p = nc.declare_dram_parameter("iota16", [16, N // 16], F32, isOutput=False)
    slio_p = nc.declare_dram_parameter("slotio", [128, CBQ], F32, isOutput=False)
    if has_rb:
        rb_p = nc.declare_dram_parameter("rbr", [128, E], F32, isOutput=False)
    if has_b2:
        b2_p = nc.declare_dram_parameter("b2r", [128, D], F32, isOutput=False)
    if has_sb2:
        sb2_p = nc.declare_dram_parameter("sb2r", [128, D], F32, isOutput=False)
    yo_p = nc.declare_dram_parameter("y_out", [NT, D], BF16, isOutput=True)
    nf_p = nc.declare_dram_parameter("nf_out", [NQ], mybir.dt.uint32, isOutput=True)

    from contextlib import ExitStack

    with tile.TileContext(nc) as tc, ExitStack() as ctx:
        ep = ctx.enter_context
        dram = ep(tc.tile_pool(name="dram", bufs=1, space="DRAM"))
        cpool = ep(tc.tile_pool(name="cpool", bufs=1))
        xslbp = ep(tc.tile_pool(name="xslbp", bufs=1))
        xgp = ep(tc.tile_pool(name="xgp", bufs=4))
        xgtp = ep(tc.tile_pool(name="xgtp", bufs=2))
        w1rp = ep(tc.tile_pool(name="w1rp", bufs=1))
        w2rp = ep(tc.tile_pool(name="w2rp", bufs=1))
        sw2rp = ep(tc.tile_pool(name="sw2rp", bufs=1))
        sw1st = ep(tc.tile_pool(name="sw1st", bufs=8))
        htp = ep(tc.tile_pool(name="htp", bufs=1))
        hshp = ep(tc.tile_pool(name="hshp", bufs=1))
        ysbp = ep(tc.tile_pool(name="ysbp", bufs=2))
        yfp = ep(tc.tile_pool(name="yfp", bufs=2))
        ldp = ep(tc.tile_pool(name="ldp", bufs=2))
        rp = ep(tc.tile_pool(name="rp", bufs=2))
        cmp_ = ep(tc.tile_pool(name="cmp", bufs=1))
        ps1 = ep(tc.tile_pool(name="ps1", bufs=2, space="PSUM"))
        ps2 = ep(tc.tile_pool(name="ps2", bufs=2, space="PSUM"))

        nc.gpsimd.load_library(library_config.sparse_gather)

        moe_q = [dram.tile([QTOK + 128, D], BF16, name=f"moe_q{q}") for q in range(NQ)]
        rs_q = [dram.tile([QTOK // NCORES, D], BF16, name=f"rs_q{q}") for q in range(NQ)]
        a2a_in = dram.tile([E, NT], F32, name="a2a_in")
        a2a_out = dram.tile([E, NT], F32, name="a2a_out")
        idx_d = [dram.tile([CAPQ, 1], F32, name=f"idx_d{q}") for q in range(NQ)]
        gate_d = [dram.tile([CAPQ, 1], F32, name=f"gate_d{q}") for q in range(NQ)]
        shared_y = dram.tile([NT, D], BF16, name="shared_y")

        # ------------- consts ---------------------------------------------
        ident = cpool.tile([128, 128], F32, name="ident")
        nc.sync.dma_start(ident[:], id_p[:])
        ident_bf = cpool.tile([128, 128], BF16, name="ident_bf")
        nc.vector.tensor_copy(ident_bf[:], ident[:])
        b1v = cpool.tile([128, NH], F32, name="b1v")
        nc.sync.dma_start(b1v[:], b1v_p[:])
        sb1v = cpool.tile([128, S * NH], F32, name="sb1v")
        nc.sync.dma_start(sb1v[:], sb1v_p[:])
        rw_sb = cpool.tile([128, ND, E], F32, name="rw_sb")
        nc.sync.dma_start(rw_sb[:], rw_p[:])
        slio = cpool.tile([128, CBQ], F32, name="slio")
        nc.scalar.dma_start(slio[:], slio_p[:])
        if has_rb:
            rbr = cpool.tile([128, E], F32, name="rbr")
            nc.sync.dma_start(rbr[:], rb_p[:])
        if has_b2:
            b2r = cpool.tile([128, D], F32, name="b2r")
            nc.sync.dma_start(b2r[:], b2_p[:])
        if has_sb2:
            sb2r = cpool.tile([128, D], F32, name="sb2r")
            nc.sync.dma_start(sb2r[:], sb2_p[:])

        # shared-expert inputs, stage2 weights, zero-fill: scalar queue
        xslb = []
        for d in range(ND):
            t = xslbp.tile([128, NT], BF16, tag=f"xslb{d}", name=f"xslb{d}")
            nc.scalar.dma_start(t[:], xslb_p[d * 128 : (d + 1) * 128, :])
            xslb.append(t)

        def load_sw2r(s):
            tiles = []
            for h in range(NH):
                t = sw2rp.tile([128, D], BF16, tag=f"sw2r{h}", name=f"sw2r{s}_{h}")
                nc.scalar.dma_start(
                    t[:], sw2_p[s * HE + h * 128 : s * HE + (h + 1) * 128, :]
                )
                tiles.append(t)
            return tiles

        sw2r = load_sw2r(0)

        zt = cpool.tile([128, D], BF16, name="zt")
        nc.vector.memset(zt[:], 0.0)
        for q in range(NQ):
            for r2 in range(QTOK // 128):
                nc.scalar.dma_start(moe_q[q][r2 * 128 : (r2 + 1) * 128, :], zt[:])

        # ------------- router: d-outer with 8 full-row loads --------------
        gall = cpool.tile([128, NT // 128, E], F32, name="gall")
        nc.vector.memset(gall[:], 0.0)
        for d in range(ND):
            xrd = rp.tile([128, NT], F32, tag="xrd", name=f"xrd{d}", bufs=1)
            nc.sync.dma_start(xrd[:], xsl_p[d * 128 : (d + 1) * 128, :])
            pgd = ps1.tile([128, NT // 128, E], F32, tag="hpsum", name=f"pgd{d}")
            for j in range(NT // 128):
                nc.tensor.matmul(
                    pgd[:, j, :],
                    xrd[:, j * 128 : (j + 1) * 128],
                    rw_sb[:, d, :],
                    start=True,
                    stop=True,
                )
            nc.vector.tensor_tensor(gall[:], gall[:], pgd[:], op=ALU.add)
        for j in range(NT // 128):
            gates = rp.tile([128, E], F32, tag="gates", name=f"gates{j}")
            if has_rb:
                nc.vector.tensor_tensor(gates[:], gall[:, j, :], rbr[:], op=ALU.add)
            else:
                nc.vector.tensor_copy(gates[:], gall[:, j, :])
            m1 = rp.tile([128, 1], F32, tag="m1", name=f"m1_{j}")
            nc.vector.tensor_reduce(m1[:], gates[:], axis=AX.X, op=ALU.max)
            mask1 = rp.tile([128, E], F32, tag="mask1", name=f"mask1_{j}")
            nc.vector.tensor_scalar(mask1[:], gates[:], m1[:], None, op0=ALU.is_equal)
            negm = rp.tile([128, E], F32, tag="negm", name=f"negm{j}")
            nc.vector.tensor_scalar(negm[:], mask1[:], -1e30, None, op0=ALU.mult)
            gm = rp.tile([128, E], F32, tag="gm", name=f"gm{j}")
            nc.vector.tensor_tensor(gm[:], gates[:], negm[:], op=ALU.add)
            m2 = rp.tile([128, 1], F32, tag="m2", name=f"m2_{j}")
            nc.vector.tensor_reduce(m2[:], gm[:], axis=AX.X, op=ALU.max)
            mask2 = rp.tile([128, E], F32, tag="mask2", name=f"mask2_{j}")
            nc.vector.tensor_scalar(mask2[:], gm[:], m2[:], None, op0=ALU.is_equal)
            dl = rp.tile([128, 1], F32, tag="dl", name=f"dl{j}")
            nc.vector.tensor_tensor(dl[:], m2[:], m1[:], op=ALU.subtract)
            e2 = rp.tile([128, 1], F32, tag="e2", name=f"e2_{j}")
            nc.scalar.activation(e2[:], dl[:], ACTF.Exp)
            den = rp.tile([128, 1], F32, tag="den", name=f"den{j}")
            nc.vector.tensor_scalar_add(den[:], e2[:], 1.0)
            p1 = rp.tile([128, 1], F32, tag="p1", name=f"p1_{j}")
            nc.vector.reciprocal(p1[:], den[:])
            p2 = rp.tile([128, 1], F32, tag="p2", name=f"p2_{j}")
            nc.vector.tensor_tensor(p2[:], e2[:], p1[:], op=ALU.mult)
            t1 = rp.tile([128, E], F32, tag="t1", name=f"t1_{j}")
            nc.vector.tensor_scalar(t1[:], mask1[:], p1[:], None, op0=ALU.mult)
            t2 = rp.tile([128, E], F32, tag="t2", name=f"t2_{j}")
            nc.vector.tensor_scalar(t2[:], mask2[:], p2[:], None, op0=ALU.mult)
            cj = rp.tile([128, E], F32, tag="cj", name=f"cj{j}")
            nc.vector.tensor_tensor(cj[:], t1[:], t2[:], op=ALU.add)
            pt = ps1.tile([E, 128], F32, tag="hpsum", name=f"pt{j}")
            nc.tensor.transpose(pt[:], cj[:], ident[:])
            cT = rp.tile([E, 128], F32, tag="cT", name=f"cT{j}", bufs=1)
            nc.vector.tensor_copy(cT[:], pt[:])
            nc.sync.dma_start(a2a_in[:, j * 128 : (j + 1) * 128], cT[:])
        nc.gpsimd.collective_compute(
            "AllToAll",
            ALU.bypass,
            replica_groups=RG,
            ins=[a2a_in.opt()],
            outs=[a2a_out.opt()],
        )

        # ------------- shared expert half-pass ----------------------------
        def shared_half(s, half):
            shts = []
            for h in range(NH):
                ph = ps1.tile([128, NT // 2], F32, tag="hpsum", name=f"shp{s}_{half}_{h}")
                for d in range(ND):
                    swt = sw1st.tile(
                        [128, 128], BF16, tag=f"sw1t{d % 2}",
                        name=f"swt{s}_{half}_{h}_{d}",
                    )
                    eng = nc.sync if d < 4 else nc.scalar
                    eng.dma_start(
                        swt[:],
                        sw1_p[
                            s * D + d * 128 : s * D + (d + 1) * 128,
                            h * 128 : (h + 1) * 128,
                        ],
                    )
                    nc.tensor.matmul(
                        ph[:],
                        swt[:],
                        xslb[d][:, half * 512 : (half + 1) * 512],
                        start=(d == 0),
                        stop=(d == ND - 1),
                    )
                sht = hshp.tile(
                    [128, NT // 2], BF16, tag=f"shT{h}", name=f"shT{s}_{half}_{h}"
                )
                nc.scalar.activation(
                    sht[:], ph[:], ACTF.Gelu, bias=sb1v[:, s * NH + h : s * NH + h + 1]
                )
                shts.append(sht)
            for kg in range(2):
                pys = []
                for ki in range(2):
                    pys.append(
                        ps2.tile(
                            [128, D], F32, tag="ypsum", name=f"spy{s}_{half}_{kg}_{ki}"
                        )
                    )
                for h in range(NH):
                    for ki in range(2):
                        for v in range(D // 512):
                            nc.tensor.matmul(
                                pys[ki][:, v * 512 : (v + 1) * 512],
                                shts[h][:, (kg * 2 + ki) * 128 : (kg * 2 + ki + 1) * 128],
                                sw2r[h][:, v * 512 : (v + 1) * 512],
                                start=(h == 0),
                                stop=(h == NH - 1),
                            )
                for ki in range(2):
                    k = half * 4 + kg * 2 + ki
                    yf = yfp.tile([128, D], BF16, tag="yf", name=f"yf{s}_{k}")
                    if s == 0:
                        nc.vector.tensor_copy(yf[:], pys[ki][:])
                    else:
                        shl = ldp.tile([128, D], BF16, tag="ldbf", name=f"shl{s}_{k}")
                        nc.sync.dma_start(shl[:], shared_y[k * 128 : (k + 1) * 128, :])
                        if has_sb2:
                            nc.vector.tensor_tensor(
                                pys[ki][:], pys[ki][:], sb2r[:], op=ALU.add
                            )
                        nc.vector.tensor_tensor(yf[:], pys[ki][:], shl[:], op=ALU.add)
                    nc.sync.dma_start(shared_y[k * 128 : (k + 1) * 128, :], yf[:])

        shared_half(0, 0)

        # routed-expert weights resident; sync transfers land in the window
        # where sync would otherwise idle waiting on the A2A
        w1r = []
        for d in range(ND):
            t = w1rp.tile([128, HE], BF16, tag=f"w1r{d}", name=f"w1r{d}")
            nc.sync.dma_start(t[:], w1_p[d * 128 : (d + 1) * 128, :])
            w1r.append(t)
        w2r = []
        for h in range(NH):
            t = w2rp.tile([128, D], BF16, tag=f"w2r{h}", name=f"w2r{h}")
            nc.sync.dma_start(t[:], w2_p[h * 128 : (h + 1) * 128, :])
            w2r.append(t)

        shared_half(0, 1)

        # ------------- per-quarter compaction + gathers -------------------
        nf_all = cpool.tile([1, NQ], mybir.dt.uint32, name="nf_all")
        idxL_sb = []
        gate_sb = []
        ix_sb = []
        xg_tiles = [[None] * CBQ for _ in range(NQ)]
        for q in range(NQ):
            c16 = cmp_.tile([16, QTOK // 16], F32, tag="c16", name=f"c16_{q}")
            for a in range(E):
                nc.sync.dma_start(
                    c16[:, a * 16 : (a + 1) * 16],
                    a2a_out[a, q * 256 : (q + 1) * 256].rearrange(
                        "(f p) -> p f", p=16
                    ),
                )
            io16q = cmp_.tile([16, QTOK // 16], F32, tag="io16q", name=f"io16q{q}", bufs=2)
            nc.sync.dma_start(io16q[:], io16_p[:, q * 128 : (q + 1) * 128])
            msk = cmp_.tile([16, QTOK // 16], F32, tag="msk", name=f"msk{q}")
            nc.vector.tensor_scalar(msk[:], c16[:], 0.0, None, op0=ALU.not_equal)
            av = cmp_.tile([16, QTOK // 16], F32, tag="av", name=f"av{q}")
            nc.vector.tensor_tensor(av[:], io16q[:], msk[:], op=ALU.mult)
            # msk -> msk-1 in place; av/ag get -1 on unselected entries
            nc.vector.tensor_scalar(msk[:], msk[:], 1.0, None, op0=ALU.subtract)
            nc.vector.tensor_tensor(av[:], av[:], msk[:], op=ALU.add)
            ag = cmp_.tile([16, QTOK // 16], F32, tag="ag", name=f"ag{q}")
            nc.vector.tensor_tensor(ag[:], c16[:], msk[:], op=ALU.add)
            idxc = cmp_.tile([16, CFQ], F32, tag="idxc", name=f"idxc{q}")
            nc.vector.memset(idxc[:], 0.0)
            nfq = cmp_.tile([1, 1], mybir.dt.uint32, tag="nfq", name=f"nfq{q}")
            nc.gpsimd.sparse_gather(idxc[:], av[:], num_found=nfq[:])
            gatec = cmp_.tile([16, CFQ], F32, tag="gatec", name=f"gatec{q}")
            nc.vector.memset(gatec[:], 0.0)
            nfq2 = cmp_.tile([1, 1], mybir.dt.uint32, tag="nfq2", name=f"nfq2{q}")
            nc.gpsimd.sparse_gather(gatec[:], ag[:], num_found=nfq2[:])
            nc.vector.tensor_copy(nf_all[:, q : q + 1], nfq[:])
            nff = cmp_.tile([1, 1], F32, tag="nff", name=f"nff{q}")
            nc.vector.tensor_copy(nff[:], nfq[:])
            nfb = cmp_.tile([128, 1], F32, tag="nfb", name=f"nfb{q}")
            nc.gpsimd.partition_broadcast(nfb[:], nff[:])
            nc.sync.dma_start(
                idx_d[q][:].rearrange("(f p) one -> p (f one)", p=16), idxc[:]
            )
            nc.sync.dma_start(
                gate_d[q][:].rearrange("(f p) one -> p (f one)", p=16), gatec[:]
            )
            idxf = cmp_.tile([128, CBQ], F32, tag="idxf", name=f"idxf{q}")
            nc.sync.dma_start(
                idxf[:], idx_d[q][:].rearrange("(c p) one -> p (c one)", p=128)
            )
            gatef = cmp_.tile([128, CBQ], F32, tag="gatef", name=f"gatef{q}")
            nc.sync.dma_start(
                gatef[:], gate_d[q][:].rearrange("(c p) one -> p (c one)", p=128)
            )
            # tail mask: slots >= nf get idx=N (skip/pad), gate=0
            imt = cmp_.tile([128, CBQ], F32, tag="imt", name=f"imt{q}")
            nc.vector.tensor_scalar(imt[:], slio[:], nfb[:], None, op0=ALU.is_lt)
            mt = cmp_.tile([128, CBQ], F32, tag="mt", name=f"mt{q}")
            nc.vector.tensor_scalar(mt[:], slio[:], nfb[:], None, op0=ALU.is_ge)
            nc.vector.tensor_tensor(gatef[:], gatef[:], imt[:], op=ALU.mult)
            nc.vector.tensor_tensor(idxf[:], idxf[:], imt[:], op=ALU.mult)
            nc.vector.tensor_scalar(mt[:], mt[:], float(N), None, op0=ALU.mult)
            nc.vector.tensor_tensor(idxf[:], idxf[:], mt[:], op=ALU.add)
            ix = cmp_.tile([128, CBQ], I32, tag=f"ix{q}", name=f"ix{q}")
            nc.vector.tensor_copy(ix[:], idxf[:])
            ixl_f = cmp_.tile([128, CBQ], F32, tag="ixlf", name=f"ixlf{q}")
            nc.vector.tensor_scalar(
                ixl_f[:], idxf[:], float(q * QTOK), None, op0=ALU.subtract
            )
            ixl = cmp_.tile([128, CBQ], I32, tag=f"ixl{q}", name=f"ixl{q}")
            nc.vector.tensor_copy(ixl[:], ixl_f[:])
            gs = cmp_.tile([128, CBQ], F32, tag=f"gs{q}", name=f"gs{q}")
            nc.vector.tensor_copy(gs[:], gatef[:])
            idxL_sb.append(ixl)
            gate_sb.append(gs)
            ix_sb.append(ix)

        def emit_gathers(q):
            for c in range(CBQ):
                xg = xgp.tile([128, D], BF16, tag="xg", name=f"xg{q}_{c}")
                if q == 0 and c < 4:
                    nc.vector.memset(xg[:], 0.0)
                nc.gpsimd.indirect_dma_start(
                    out=xg[:],
                    out_offset=None,
                    in_=xtm_p[:],
                    in_offset=bass.IndirectOffsetOnAxis(
                        ap=ix_sb[q][:, c : c + 1], axis=0
                    ),
                    bounds_check=N - 1,
                    oob_is_err=False,
                )
                xg_tiles[q][c] = xg

        # PE transposes of gathered tokens, drained to SBUF by vector
        def emit_transposes(q):
            xgt = []
            for d in range(ND):
                t = xgtp.tile([128, CAPQ], BF16, tag=f"xgt{d}", name=f"xgt{q}_{d}")
                xgt.append(t)
            for c in range(CBQ):
                for d in range(ND):
                    tp = ps1.tile([128, 128], BF16, tag="hpsum", name=f"tp{q}_{c}_{d}")
                    nc.tensor.transpose(
                        tp[:], xg_tiles[q][c][:, d * 128 : (d + 1) * 128], ident_bf[:]
                    )
                    nc.vector.tensor_copy(xgt[d][:, c * 128 : (c + 1) * 128], tp[:])
            return xgt

        emit_gathers(0)

        # ------------- routed quarters: dense tensor stream ---------------
        def quarter_s1(q):
            xgt = xgt_q[q]
            hts = []
            for h in range(NH):
                ph = ps1.tile([128, CAPQ], F32, tag="hpsum", name=f"ph{q}_{h}")
                for d in range(ND):
                    nc.tensor.matmul(
                        ph[:, 0:512],
                        w1r[d][:, h * 128 : (h + 1) * 128],
                        xgt[d][:, 0:512],
                        start=(d == 0),
                        stop=(d == ND - 1),
                    )
                    nc.tensor.matmul(
                        ph[:, 512:CAPQ],
                        w1r[d][:, h * 128 : (h + 1) * 128],
                        xgt[d][:, 512:CAPQ],
                        start=(d == 0),
                        stop=(d == ND - 1),
                    )
                ht = htp.tile([128, CAPQ], BF16, tag=f"hT{h}", name=f"hTe{q}_{h}")
                nc.scalar.activation(ht[:], ph[:], ACTF.Gelu, bias=b1v[:, h : h + 1])
                hts.append(ht)
            return hts

        def quarter_s2(q, hts):
            for c in range(CBQ):
                py = ps2.tile([128, D], F32, tag="ypsum", name=f"py{q}_{c}")
                for h in range(NH):
                    for v in range(D // 512):
                        nc.tensor.matmul(
                            py[:, v * 512 : (v + 1) * 512],
                            hts[h][:, c * 128 : (c + 1) * 128],
                            w2r[h][:, v * 512 : (v + 1) * 512],
                            start=(h == 0),
                            stop=(h == NH - 1),
                        )
                if has_b2:
                    nc.vector.tensor_tensor(py[:], py[:], b2r[:], op=ALU.add)
                ysb = ysbp.tile([128, D], BF16, tag="ysb", name=f"ysbq{q}_{c}")
                nc.vector.tensor_scalar(
                    ysb[:], py[:], gate_sb[q][:, c : c + 1], None, op0=ALU.mult
                )
                nc.gpsimd.indirect_dma_start(
                    out=moe_q[q][:],
                    out_offset=bass.IndirectOffsetOnAxis(
                        ap=idxL_sb[q][:, c : c + 1], axis=0
                    ),
                    in_=ysb[:],
                    in_offset=None,
                    bounds_check=QTOK + 127,
                    oob_is_err=False,
                )
            nc.gpsimd.collective_compute(
                "ReduceScatter",
                ALU.add,
                replica_groups=RG,
                ins=[moe_q[q][0:QTOK, :].opt()],
                outs=[rs_q[q].opt()],
            )

        def rs_final(q):
            for r in range(2):
                k = q * 2 + r
                rl = ldp.tile([128, D], BF16, tag="ldbf", name=f"rsld{k}")
                nc.sync.dma_start(rl[:], rs_q[q][r * 128 : (r + 1) * 128, :])
                sl = ldp.tile([128, D], BF16, tag="ldbf", name=f"shld{k}")
                nc.sync.dma_start(sl[:], shared_y[k * 128 : (k + 1) * 128, :])
                fo = yfp.tile([128, D], BF16, tag="yf", name=f"fo{k}")
                nc.vector.tensor_tensor(fo[:], rl[:], sl[:], op=ALU.add)
                nc.sync.dma_start(yo_p[k * 128 : (k + 1) * 128, :], fo[:])

        xgt_q = {}
        xgt_q[0] = emit_transposes(0)
        emit_gathers(1)
        hts0 = quarter_s1(0)
        quarter_s2(0, hts0)
        sw2r = load_sw2r(1)
        shared_half(1, 0)
        shared_half(1, 1)
        rs_final(0)
        xgt_q[1] = emit_transposes(1)
        emit_gathers(2)
        hts1 = quarter_s1(1)
        quarter_s2(1, hts1)
        rs_final(1)
        xgt_q[2] = emit_transposes(2)
        emit_gathers(3)
        hts2 = quarter_s1(2)
        quarter_s2(2, hts2)
        rs_final(2)
        xgt_q[3] = emit_transposes(3)
        hts3 = quarter_s1(3)
        quarter_s2(3, hts3)
        rs_final(3)

        nc.sync.dma_start(nf_p[:], nf_all[:])

    nc.compile()
    return nc


def _get_nc(key):
    if key not in _NC_CACHE:
        _NC_CACHE[key] = _build_sparse(*key)
    return _NC_CACHE[key]


def _get_nc_dense(key):
    k2 = ("dense",) + key
    if k2 not in _NC_CACHE:
        _NC_CACHE[k2] = _build(*key)
    return _NC_CACHE[k2]


def _prep_in_maps(x, router_w, router_b, w1, b1, w2, b2, sw1, sb1, sw2, sb2,
                  sparse=True):
    f32 = np.float32
    x2 = np.ascontiguousarray(np.asarray(x, f32).reshape(N, D))
    x2bf = x2.astype(ml_dtypes.bfloat16)
    xt = np.ascontiguousarray(x2.T)

    has_rb = bool(np.any(router_b))
    has_b2 = bool(np.any(b2))
    sb2_eff = np.asarray(sb2, f32).sum(0) / S
    has_sb2 = bool(np.any(sb2_eff))
    key = (has_rb, has_b2, has_sb2)

    rw_r = np.ascontiguousarray(
        np.asarray(router_w, f32).reshape(ND, 128, E).transpose(1, 0, 2)
    )
    ident = np.eye(128, dtype=f32)
    # sparse variant: core e owns 256-token blocks {b : b % 8 == e}.
    # iota16[p, q*128 + a*16 + f] = global id of a2a-flat position
    # s' = a*256 + f*16 + p within quarter q = (a + 8q)*256 + f*16 + p
    iota16 = np.empty((16, N // 16), f32)
    for q in range(4):
        for a in range(E):
            for f in range(16):
                col = q * 128 + a * 16 + f
                iota16[:, col] = (a + 8 * q) * 256 + f * 16 + np.arange(16)
    slotio = (
        np.arange(CAPQ_HOST // 128)[None, :] * 128 + np.arange(128)[:, None]
    ).astype(f32)
    sw1s = np.ascontiguousarray(np.asarray(sw1, f32).reshape(S * D, HE))
    sw1s_bf = sw1s.astype(ml_dtypes.bfloat16)
    sw2s = np.ascontiguousarray(
        (np.asarray(sw2, f32) * (1.0 / S)).reshape(S * HE, D)
    ).astype(ml_dtypes.bfloat16)
    sb1v = np.ascontiguousarray(
        np.asarray(sb1, f32).reshape(S, NH, 128).transpose(2, 0, 1).reshape(128, S * NH)
    )
    if has_rb:
        rbr = np.tile(np.asarray(router_b, f32), (128, 1))
    if has_sb2:
        sb2r = np.tile(sb2_eff, (128, 1))

    in_maps = []
    for e in range(NCORES):
        if sparse:
            own = np.concatenate(
                [x2[(q * 8 + e) * 256 : (q * 8 + e + 1) * 256] for q in range(4)]
            )
            xsl_e = np.ascontiguousarray(own.T)
        else:
            xsl_e = np.ascontiguousarray(xt[:, e * NT : (e + 1) * NT])
        m = {
            "xsl": xsl_e,
            "w1": np.ascontiguousarray(np.asarray(w1[e], f32)),
            "w2": np.ascontiguousarray(np.asarray(w2[e], f32)).astype(ml_dtypes.bfloat16),
            "rw": rw_r,
            "sw1": sw1s,
            "sw2": sw2s,
            "b1v": np.ascontiguousarray(np.asarray(b1[e], f32).reshape(NH, 128).T),
            "sb1v": sb1v,
            "ident": ident,
        }
        if sparse:
            m["xtm"] = x2bf
            m["xslb"] = xsl_e.astype(ml_dtypes.bfloat16)
            m["iota16"] = iota16
            m["slotio"] = slotio
            m["w1"] = m["w1"].astype(ml_dtypes.bfloat16)
            m["sw1"] = sw1s_bf
        else:
            m["xt"] = xt
        if has_rb:
            m["rbr"] = rbr
        if has_b2:
            m["b2r"] = np.tile(np.asarray(b2[e], f32), (128, 1))
        if has_sb2:
            m["sb2r"] = sb2r
        in_maps.append(m)
    return key, in_maps


def _install_ntff_hook():
    """Re-create the boot-time NTFF profile hook (this image's antenv lacks
    axon_hooks, so trn_boot degraded silently). Needed only for tracing."""
    import contextlib
    import ctypes
    import types

    try:
        from antenv.axon_hooks import get_axon_ntff_profile_hook  # noqa: F401

        return
    except ImportError:
        pass

    so_path = "/opt/axon/libaxon_pjrt.so"
    lib = ctypes.CDLL(so_path)
    if not hasattr(lib, "axon_start_nrt_profile"):
        return
    lib.axon_start_nrt_profile.argtypes = [
        ctypes.POINTER(ctypes.c_int64),
        ctypes.c_size_t,
    ]
    lib.axon_start_nrt_profile.restype = ctypes.c_int64
    lib.axon_stop_nrt_profile.argtypes = [ctypes.c_char_p]
    lib.axon_stop_nrt_profile.restype = ctypes.c_int64

    @contextlib.contextmanager
    def _hook(output_dir, device_ids):
        import jax

        jax.devices()
        if device_ids:
            ids = (ctypes.c_int64 * len(device_ids))(*device_ids)
            rc = lib.axon_start_nrt_profile(ids, len(device_ids))
        else:
            rc = lib.axon_start_nrt_profile(None, 0)
        if rc != 0:
            raise RuntimeError(f"axon_start_nrt_profile rc={rc}")
        try:
            yield
        finally:
            n = lib.axon_stop_nrt_profile(str(output_dir).encode())
            print(f"profile: {n} file(s) written to {output_dir}", file=sys.stderr)

    mod = types.ModuleType("antenv.axon_hooks")
    mod.get_axon_ntff_profile_hook = lambda: _hook
    mod.set_axon_ntff_profile_hook = lambda h: None
    sys.modules["antenv.axon_hooks"] = mod


def kernel(x, router_w, router_b, w1, b1, w2, b2, sw1, sb1, sw2, sb2, _trace=False):
    if _trace:
        _install_ntff_hook()
    args = (x, router_w, router_b, w1, b1, w2, b2, sw1, sb1, sw2, sb2)
    key, in_maps = _prep_in_maps(*args, sparse=True)
    nc = _get_nc(key)
    res = run_bass_kernel_spmd(
        nc, in_maps, core_ids=list(range(NCORES)), trace=_trace
    )
    counts = [int(c) for e in range(NCORES) for c in res.results[e]["nf_out"]]
    out = np.empty((N, D), np.float32)
    if max(counts) > CAPQ_HOST:
        # capacity overflow (pathologically imbalanced routing):
        # fall back to the dense variant, which is correct for any routing
        key, in_maps = _prep_in_maps(*args, sparse=False)
        nc = _get_nc_dense(key)
        res = run_bass_kernel_spmd(
            nc, in_maps, core_ids=list(range(NCORES)), trace=_trace
        )
        for e in range(NCORES):
            out[e * NT : (e + 1) * NT] = res.results[e]["y_out"]
    else:
        for e in range(NCORES):
            yo = np.asarray(res.results[e]["y_out"], np.float32)
            for q in range(4):
                out[(q * 8 + e) * 256 : (q * 8 + e + 1) * 256] = yo[
                    q * 256 : (q + 1) * 256
                ]
    out = out.reshape(np.asarray(x).shape)
    if _trace:
        return out, res
    return out



# revision 39
# speedup vs baseline: 1.1167x; 1.1167x over previous
"""MoE feed-forward (8 experts, top-2, 2 shared experts) on 8 TRN2 NeuronCores.

Strategy (expert-parallel):
  - 1 expert per core. Router computed on-device per core for its own 1/8
    token slice (token-major), then a tiny AllToAll distributes comb columns
    so core e ends up with gate weights of expert e for ALL tokens.
  - Dense expert FFN per core in fp32r (stage1, feature-major hidden) /
    bf16 (stage2, token-major output). Gate scaling is a per-partition
    tensor_scalar on the token-major output.
  - ReduceScatter sums expert contributions across cores and hands each
    core its own token slice.
  - Shared experts are computed data-parallel (each core: its token slice),
    overlapping the ReduceScatter, and added locally before writing out.
"""

import sys

if "/opt/trn_rl_repo" not in sys.path:
    sys.path.insert(0, "/opt/trn_rl_repo")

import numpy as np
import ml_dtypes

import concourse.bass as bass
import concourse.tile as tile
import concourse.mybir as mybir
from concourse import bacc
from concourse.bass_utils import run_bass_kernel_spmd

F32 = mybir.dt.float32
F32R = mybir.dt.float32r
BF16 = mybir.dt.bfloat16
I32 = mybir.dt.int32
AX = mybir.AxisListType
ALU = mybir.AluOpType
ACTF = mybir.ActivationFunctionType

N, D, HE, E, S = 8192, 1024, 2048, 8, 2
NCORES = 8
NT = N // NCORES      # 1024 tokens per core slice
TBC = 1024            # token chunk for the dense expert stages
NTBC = N // TBC
ND = D // 128         # 8
NH = HE // 128        # 16
RG = [list(range(NCORES))]

CAPQ_HOST = 640       # sparse: per-(expert, quarter) slot capacity

_NC_CACHE = {}


def _build(has_rb, has_b2, has_sb2, debug=False):
    nc = bacc.Bacc(None, target_bir_lowering=False)

    xt_p = nc.declare_dram_parameter("xt", [D, N], F32R, isOutput=False)
    xsl_p = nc.declare_dram_parameter("xsl", [D, NT], F32R, isOutput=False)
    w1_p = nc.declare_dram_parameter("w1", [D, HE], F32R, isOutput=False)
    w2_p = nc.declare_dram_parameter("w2", [HE, D], BF16, isOutput=False)
    rw_p = nc.declare_dram_parameter("rw", [128, ND, E], F32R, isOutput=False)
    sw1_p = nc.declare_dram_parameter("sw1", [S * D, HE], F32R, isOutput=False)
    sw2_p = nc.declare_dram_parameter("sw2", [S * HE, D], BF16, isOutput=False)
    b1v_p = nc.declare_dram_parameter("b1v", [128, NH], F32, isOutput=False)
    sb1v_p = nc.declare_dram_parameter("sb1v", [128, S * NH], F32, isOutput=False)
    id_p = nc.declare_dram_parameter("ident", [128, 128], F32, isOutput=False)
    if has_rb:
        rb_p = nc.declare_dram_parameter("rbr", [128, E], F32, isOutput=False)
    if has_b2:
        b2_p = nc.declare_dram_parameter("b2r", [128, D], F32, isOutput=False)
    if has_sb2:
        sb2_p = nc.declare_dram_parameter("sb2r", [128, D], F32, isOutput=False)
    yo_p = nc.declare_dram_parameter("y_out", [NT, D], F32, isOutput=True)
    if debug:
        dbg_combT_p = nc.declare_dram_parameter("dbg_combT", [E, NT], F32, isOutput=True)
        dbg_ctm_p = nc.declare_dram_parameter("dbg_ctm", [128, N // 128], F32, isOutput=True)
        dbg_ysh_p = nc.declare_dram_parameter("dbg_ysh", [NT, D], F32, isOutput=True)

    from contextlib import ExitStack

    with tile.TileContext(nc) as tc, ExitStack() as ctx:
        ep = ctx.enter_context
        dram = ep(tc.tile_pool(name="dram", bufs=1, space="DRAM"))
        cpool = ep(tc.tile_pool(name="cpool", bufs=1))
        xslp = ep(tc.tile_pool(name="xslp", bufs=1))
        xtp = ep(tc.tile_pool(name="xtp", bufs=1))
        htp = ep(tc.tile_pool(name="htp", bufs=1))
        w2rp = ep(tc.tile_pool(name="w2rp", bufs=1))
        wst = ep(tc.tile_pool(name="wst", bufs=6))
        sw2st = ep(tc.tile_pool(name="sw2st", bufs=3))
        ysbp = ep(tc.tile_pool(name="ysbp", bufs=2))
        finp = ep(tc.tile_pool(name="finp", bufs=2))
        rp = ep(tc.tile_pool(name="rp", bufs=2))
        ps1 = ep(tc.tile_pool(name="ps1", bufs=2, space="PSUM"))
        ps2 = ep(tc.tile_pool(name="ps2", bufs=2, space="PSUM"))

        moe_y = dram.tile([N, D], F32, name="moe_y")
        rs_out = dram.tile([NT, D], F32, name="rs_out")
        a2a_in = dram.tile([E, NT], F32, name="a2a_in")
        a2a_out = dram.tile([E, NT], F32, name="a2a_out")

        ident = cpool.tile([128, 128], F32, name="ident")
        nc.sync.dma_start(ident[:], id_p[:])
        b1v = cpool.tile([128, NH], F32, name="b1v")
        nc.sync.dma_start(b1v[:], b1v_p[:])
        sb1v = cpool.tile([128, S * NH], F32, name="sb1v")
        nc.sync.dma_start(sb1v[:], sb1v_p[:])
        # router path in true fp32 tiles (PE matmul mode follows the SBUF
        # tensor dtype; fp32r noise ~3e-4 exceeds the smallest gate gap 2e-5)
        rw_sb = cpool.tile([128, ND, E], F32, name="rw_sb")
        nc.sync.dma_start(rw_sb[:], rw_p[:].bitcast(F32))
        if has_rb:
            rbr = cpool.tile([128, E], F32, name="rbr")
            nc.sync.dma_start(rbr[:], rb_p[:])
        if has_b2:
            b2r = cpool.tile([128, D], F32, name="b2r")
            nc.sync.dma_start(b2r[:], b2_p[:])
        if has_sb2:
            sb2r = cpool.tile([128, D], F32, name="sb2r")
            nc.sync.dma_start(sb2r[:], sb2_p[:])

        xsl = []
        for d in range(ND):
            t = xslp.tile([128, NT], F32R, tag=f"xsl{d}", name=f"xsl{d}")
            nc.sync.dma_start(t[:], xsl_p[d * 128 : (d + 1) * 128, :])
            xsl.append(t)

        # ---------------- router (own token slice, token-major) ----------
        combT = cpool.tile([E, NT], F32, name="combT")
        for j in range(NT // 128):
            pg = ps1.tile([128, E], F32, tag="hpsum", name=f"pg{j}")
            for d in range(ND):
                xr = rp.tile([128, 128], F32, tag="xr", name=f"xr{j}_{d}", bufs=4)
                nc.sync.dma_start(
                    xr[:],
                    xsl_p[d * 128 : (d + 1) * 128, j * 128 : (j + 1) * 128].bitcast(F32),
                )
                nc.tensor.matmul(
                    pg[:],
                    xr[:],
                    rw_sb[:, d, :],
                    start=(d == 0),
                    stop=(d == ND - 1),
                )
            gates = rp.tile([128, E], F32, tag="gates", name=f"gates{j}")
            if has_rb:
                nc.vector.tensor_tensor(gates[:], pg[:], rbr[:], op=ALU.add)
            else:
                nc.vector.tensor_copy(gates[:], pg[:])
            m1 = rp.tile([128, 1], F32, tag="m1", name=f"m1_{j}")
            nc.vector.tensor_reduce(m1[:], gates[:], axis=AX.X, op=ALU.max)
            mask1 = rp.tile([128, E], F32, tag="mask1", name=f"mask1_{j}")
            nc.vector.tensor_scalar(mask1[:], gates[:], m1[:], None, op0=ALU.is_equal)
            negm = rp.tile([128, E], F32, tag="negm", name=f"negm{j}")
            nc.vector.tensor_scalar(negm[:], mask1[:], -1e30, None, op0=ALU.mult)
            gm = rp.tile([128, E], F32, tag="gm", name=f"gm{j}")
            nc.vector.tensor_tensor(gm[:], gates[:], negm[:], op=ALU.add)
            m2 = rp.tile([128, 1], F32, tag="m2", name=f"m2_{j}")
            nc.vector.tensor_reduce(m2[:], gm[:], axis=AX.X, op=ALU.max)
            mask2 = rp.tile([128, E], F32, tag="mask2", name=f"mask2_{j}")
            nc.vector.tensor_scalar(mask2[:], gm[:], m2[:], None, op0=ALU.is_equal)
            dl = rp.tile([128, 1], F32, tag="dl", name=f"dl{j}")
            nc.vector.tensor_tensor(dl[:], m2[:], m1[:], op=ALU.subtract)
            e2 = rp.tile([128, 1], F32, tag="e2", name=f"e2_{j}")
            nc.scalar.activation(e2[:], dl[:], ACTF.Exp)
            den = rp.tile([128, 1], F32, tag="den", name=f"den{j}")
            nc.vector.tensor_scalar_add(den[:], e2[:], 1.0)
            p1 = rp.tile([128, 1], F32, tag="p1", name=f"p1_{j}")
            nc.vector.reciprocal(p1[:], den[:])
            p2 = rp.tile([128, 1], F32, tag="p2", name=f"p2_{j}")
            nc.vector.tensor_tensor(p2[:], e2[:], p1[:], op=ALU.mult)
            t1 = rp.tile([128, E], F32, tag="t1", name=f"t1_{j}")
            nc.vector.tensor_scalar(t1[:], mask1[:], p1[:], None, op0=ALU.mult)
            t2 = rp.tile([128, E], F32, tag="t2", name=f"t2_{j}")
            nc.vector.tensor_scalar(t2[:], mask2[:], p2[:], None, op0=ALU.mult)
            cj = rp.tile([128, E], F32, tag="cj", name=f"cj{j}")
            nc.vector.tensor_tensor(cj[:], t1[:], t2[:], op=ALU.add)
            pt = ps1.tile([E, 128], F32, tag="hpsum", name=f"pt{j}")
            nc.tensor.transpose(pt[:], cj[:], ident[:])
            nc.vector.tensor_copy(combT[:, j * 128 : (j + 1) * 128], pt[:])
        nc.sync.dma_start(a2a_in[:], combT[:])
        nc.gpsimd.collective_compute(
            "AllToAll",
            ALU.bypass,
            replica_groups=RG,
            ins=[a2a_in.opt()],
            outs=[a2a_out.opt()],
        )
        comb_tm = cpool.tile([128, N // 128], F32, name="comb_tm")
        nc.sync.dma_start(
            comb_tm[:], a2a_out[:].rearrange("a (c p) -> p (a c)", p=128)
        )
        if debug:
            nc.sync.dma_start(dbg_combT_p[:], combT[:])
            nc.sync.dma_start(dbg_ctm_p[:], comb_tm[:])

        # ---------------- dense expert FFN ------------------------------
        w2res = []
        for h in range(NH):
            t = w2rp.tile([128, D], BF16, tag=f"w2r{h}", name=f"w2r{h}")
            nc.sync.dma_start(t[:], w2_p[h * 128 : (h + 1) * 128, :])
            w2res.append(t)

        for tb in range(NTBC):
            xts = []
            for d in range(ND):
                t = xtp.tile([128, TBC], F32R, tag=f"xt{d}", name=f"xt{tb}_{d}")
                nc.sync.dma_start(t[:], xt_p[d * 128 : (d + 1) * 128, tb * TBC : (tb + 1) * TBC])
                xts.append(t)
            hts = []
            for h in range(NH):
                ph = ps1.tile([128, TBC], F32, tag="hpsum", name=f"ph{tb}_{h}")
                for d in range(ND):
                    w1t = wst.tile([128, 128], F32R, tag="w1t", name=f"w1t{tb}_{h}_{d}")
                    nc.sync.dma_start(w1t[:], w1_p[d * 128 : (d + 1) * 128, h * 128 : (h + 1) * 128])
                    for v in range(TBC // 512):
                        nc.tensor.matmul(
                            ph[:, v * 512 : (v + 1) * 512],
                            w1t[:],
                            xts[d][:, v * 512 : (v + 1) * 512],
                            start=(d == 0),
                            stop=(d == ND - 1),
                        )
                ht = htp.tile([128, TBC], BF16, tag=f"hT{h}", name=f"hT{tb}_{h}")
                nc.scalar.activation(ht[:], ph[:], ACTF.Gelu, bias=b1v[:, h : h + 1])
                hts.append(ht)
            for k in range(TBC // 128):
                g = tb * (TBC // 128) + k
                py = ps2.tile([128, D], F32, tag="ypsum", name=f"py{g}")
                for h in range(NH):
                    for v in range(D // 512):
                        nc.tensor.matmul(
                            py[:, v * 512 : (v + 1) * 512],
                            hts[h][:, k * 128 : (k + 1) * 128],
                            w2res[h][:, v * 512 : (v + 1) * 512],
                            start=(h == 0),
                            stop=(h == NH - 1),
                        )
                if has_b2:
                    nc.vector.tensor_tensor(py[:], py[:], b2r[:], op=ALU.add)
                ysb = ysbp.tile([128, D], F32, tag="ysb", name=f"ysb{g}")
                nc.vector.tensor_scalar(ysb[:], py[:], comb_tm[:, g : g + 1], None, op0=ALU.mult)
                nc.sync.dma_start(moe_y[g * 128 : (g + 1) * 128, :], ysb[:])

        # ---------------- combine across cores --------------------------
        nc.gpsimd.collective_compute(
            "ReduceScatter",
            ALU.add,
            replica_groups=RG,
            ins=[moe_y.opt()],
            outs=[rs_out.opt()],
        )

        # ---------------- shared experts (overlap the RS) ----------------
        ysh = []
        for k in range(NT // 128):
            t = xtp.tile([128, D], F32, tag=f"xt{k}", name=f"ysh{k}")
            ysh.append(t)
        for s in range(S):
            shts = []
            for h in range(NH):
                ph = ps1.tile([128, NT], F32, tag="hpsum", name=f"shp{s}_{h}")
                for d in range(ND):
                    swt = wst.tile([128, 128], F32R, tag="w1t", name=f"swt{s}_{h}_{d}")
                    nc.sync.dma_start(swt[:], sw1_p[s * D + d * 128 : s * D + (d + 1) * 128, h * 128 : (h + 1) * 128])
                    for v in range(NT // 512):
                        nc.tensor.matmul(
                            ph[:, v * 512 : (v + 1) * 512],
                            swt[:],
                            xsl[d][:, v * 512 : (v + 1) * 512],
                            start=(d == 0),
                            stop=(d == ND - 1),
                        )
                sht = htp.tile([128, NT], BF16, tag=f"hT{h}", name=f"shT{s}_{h}")
                nc.scalar.activation(sht[:], ph[:], ACTF.Gelu, bias=sb1v[:, s * NH + h : s * NH + h + 1])
                shts.append(sht)
            for kg in range(NT // 256):
                pys = []
                for ki in range(2):
                    k = kg * 2 + ki
                    pys.append(ps2.tile([128, D], F32, tag="ypsum", name=f"spy{s}_{k}"))
                for h in range(NH):
                    sw2t = sw2st.tile([128, D], BF16, tag="sw2t", name=f"sw2t{s}_{kg}_{h}")
                    nc.sync.dma_start(sw2t[:], sw2_p[s * HE + h * 128 : s * HE + (h + 1) * 128, :])
                    for ki in range(2):
                        k = kg * 2 + ki
                        for v in range(D // 512):
                            nc.tensor.matmul(
                                pys[ki][:, v * 512 : (v + 1) * 512],
                                shts[h][:, k * 128 : (k + 1) * 128],
                                sw2t[:, v * 512 : (v + 1) * 512],
                                start=(h == 0),
                                stop=(h == NH - 1),
                            )
                for ki in range(2):
                    k = kg * 2 + ki
                    if s == 0:
                        nc.vector.tensor_copy(ysh[k][:], pys[ki][:])
                    else:
                        nc.vector.tensor_tensor(ysh[k][:], ysh[k][:], pys[ki][:], op=ALU.add)

        # ---------------- final: rs slice + shared ------------------------
        for k in range(NT // 128):
            fin = finp.tile([128, D], F32, tag="fin", name=f"fin{k}")
            if debug:
                nc.sync.dma_start(dbg_ysh_p[k * 128 : (k + 1) * 128, :], ysh[k][:])
            nc.sync.dma_start(fin[:], rs_out[k * 128 : (k + 1) * 128, :])
            nc.vector.tensor_tensor(fin[:], fin[:], ysh[k][:], op=ALU.add)
            if has_sb2:
                nc.vector.tensor_tensor(fin[:], fin[:], sb2r[:], op=ALU.add)
            nc.sync.dma_start(yo_p[k * 128 : (k + 1) * 128, :], fin[:])

    nc.compile()
    return nc


def _build_sparse(has_rb, has_b2, has_sb2, debug=False):
    """Optimized sparse expert-parallel kernel (v3).

    vs v2: router matmuls keep the router weights stationary (free dim 512
    instead of 8), the top-2/softmax chain is batched into ~13 vector ops
    over [128, 8, 8], gathered tokens arrive pre-transposed via one
    dma_gather(transpose=True) per quarter (no PE transposes), shared
    weights stream as column-blocks/full rows (few large DMAs), the
    moe zero-fill rides the sync queue behind the bulk loads, and the
    last shared-expert pass runs at the end to overlap the final RS.
    """
    nc = bacc.Bacc(None, target_bir_lowering=False)

    NQ = 4                 # token quarters (chunked ReduceScatter)
    QTOK = N // NQ         # 2048 tokens per quarter
    CAPQ = 640             # per-(expert, quarter) slot capacity (max seen 559)
    CBQ = CAPQ // 128      # 5 slot blocks per quarter
    CFQ = CAPQ // 16       # sparse_gather out free dim

    xtm_p = nc.declare_dram_parameter("xtm", [N, D], BF16, isOutput=False)
    xsl_p = nc.declare_dram_parameter("xsl", [D, NT], F32, isOutput=False)
    xslb_p = nc.declare_dram_parameter("xslb", [D, NT], BF16, isOutput=False)
    w1_p = nc.declare_dram_parameter("w1", [D, HE], BF16, isOutput=False)
    w2_p = nc.declare_dram_parameter("w2", [HE, D], BF16, isOutput=False)
    rw_p = nc.declare_dram_parameter("rw", [128, ND, E], F32, isOutput=False)
    sw1_p = nc.declare_dram_parameter("sw1", [S * D, HE], BF16, isOutput=False)
    sw2_p = nc.declare_dram_parameter("sw2", [S * HE, D], BF16, isOutput=False)
    b1v_p = nc.declare_dram_parameter("b1v", [128, NH], F32, isOutput=False)
    sb1v_p = nc.declare_dram_parameter("sb1v", [128, S * NH], F32, isOutput=False)
    id_p = nc.declare_dram_parameter("ident", [128, 128], F32, isOutput=False)
    io16_p = nc.declare_dram_parameter("iota16", [16, N // 16], F32, isOutput=False)
    rep16_p = nc.declare_dram_parameter("rep16", [16, 128], F32, isOutput=False)
    sl16_p = nc.declare_dram_parameter("slot16", [16, CFQ], F32, isOutput=False)
    if has_rb:
        rb_p = nc.declare_dram_parameter("rbr", [128, E], F32, isOutput=False)
    if has_b2:
        b2_p = nc.declare_dram_parameter("b2r", [128, D], F32, isOutput=False)
    if has_sb2:
        sb2_p = nc.declare_dram_parameter("sb2r", [128, D], F32, isOutput=False)
    yo_p = nc.declare_dram_parameter("y_out", [NT, D], BF16, isOutput=True)
    nf_p = nc.declare_dram_parameter("nf_out", [NQ], mybir.dt.uint32, isOutput=True)
    if debug:
        dbg_comb_p = nc.declare_dram_parameter(
            "dbg_comb", [128, (NT // 128) * E], F32, isOutput=True
        )
        dbg_a2a_p = nc.declare_dram_parameter("dbg_a2a", [E, NT], F32, isOutput=True)
        dbg_gs_p = nc.declare_dram_parameter("dbg_gs", [128, NQ * CBQ], F32, isOutput=True)
        dbg_ixl_p = nc.declare_dram_parameter("dbg_ixl", [128, NQ * CBQ], F32, isOutput=True)
        dbg_xgt_p = nc.declare_dram_parameter("dbg_xgt", [128, ND * CAPQ], BF16, isOutput=True)
        dbg_ht_p = nc.declare_dram_parameter("dbg_ht", [128, CAPQ], BF16, isOutput=True)
        dbg_ysb_p = nc.declare_dram_parameter("dbg_ysb", [128, D], BF16, isOutput=True)
        dbg_moe_p = nc.declare_dram_parameter("dbg_moe", [QTOK, D], BF16, isOutput=True)
        dbg_rs_p = nc.declare_dram_parameter("dbg_rs", [QTOK // NCORES, D], BF16, isOutput=True)
        dbg_av_p = nc.declare_dram_parameter("dbg_av", [16, NQ * 128], F32, isOutput=True)
        dbg_ag_p = nc.declare_dram_parameter("dbg_ag", [16, NQ * 128], F32, isOutput=True)
        dbg_idxc_p = nc.declare_dram_parameter("dbg_idxc", [16, NQ * CFQ], F32, isOutput=True)
        dbg_gatec_p = nc.declare_dram_parameter("dbg_gatec", [16, NQ * CFQ], F32, isOutput=True)
        dbg_ixm_p = nc.declare_dram_parameter("dbg_ixm", [16, NQ * CFQ], F32, isOutput=True)
        dbg_ix16_p = nc.declare_dram_parameter("dbg_ix16", [128, NQ * CFQ], mybir.dt.int16, isOutput=True)

    from contextlib import ExitStack

    with tile.TileContext(nc) as tc, ExitStack() as ctx:
        ep = ctx.enter_context
        dram = ep(tc.tile_pool(name="dram", bufs=1, space="DRAM"))
        cpool = ep(tc.tile_pool(name="cpool", bufs=1))
        xslbp = ep(tc.tile_pool(name="xslbp", bufs=1))
        xgtp = ep(tc.tile_pool(name="xgtp", bufs=2))
        w1rp = ep(tc.tile_pool(name="w1rp", bufs=1))
        w2rp = ep(tc.tile_pool(name="w2rp", bufs=1))
        sw2rp = ep(tc.tile_pool(name="sw2rp", bufs=1))
        sw1cp = ep(tc.tile_pool(name="sw1cp", bufs=3))
        htp = ep(tc.tile_pool(name="htp", bufs=1))
        hshp = ep(tc.tile_pool(name="hshp", bufs=1))
        ysbp = ep(tc.tile_pool(name="ysbp", bufs=4))
        yfp = ep(tc.tile_pool(name="yfp", bufs=2))
        ldp = ep(tc.tile_pool(name="ldp", bufs=2))
        rp = ep(tc.tile_pool(name="rp", bufs=2))
        cmp_ = ep(tc.tile_pool(name="cmp", bufs=1))
        ps1 = ep(tc.tile_pool(name="ps1", bufs=2, space="PSUM"))
        ps2 = ep(tc.tile_pool(name="ps2", bufs=2, space="PSUM"))

        moe_q = [dram.tile([QTOK + 128, D], BF16, name=f"moe_q{q}") for q in range(NQ)]
        rs_q = [dram.tile([QTOK // NCORES, D], BF16, name=f"rs_q{q}") for q in range(NQ)]
        a2a_in = dram.tile([E, NT], F32, name="a2a_in")
        a2a_out = dram.tile([E, NT], F32, name="a2a_out")
        idx_d = [dram.tile([CAPQ, 1], F32, name=f"idx_d{q}") for q in range(NQ)]
        gate_d = [dram.tile([CAPQ, 1], F32, name=f"gate_d{q}") for q in range(NQ)]
        shared_y = dram.tile([NT, D], BF16, name="shared_y")

        # ------------- consts ---------------------------------------------
        ident = cpool.tile([128, 128], F32, name="ident")
        nc.sync.dma_start(ident[:], id_p[:])
        b1v = cpool.tile([128, NH], F32, name="b1v")
        nc.sync.dma_start(b1v[:], b1v_p[:])
        sb1v = cpool.tile([128, S * NH], F32, name="sb1v")
        nc.sync.dma_start(sb1v[:], sb1v_p[:])
        rw_sb = cpool.tile([128, ND, E], F32, name="rw_sb")
        nc.sync.dma_start(rw_sb[:], rw_p[:])
        rep16 = cpool.tile([16, 128], F32, name="rep16")
        nc.sync.dma_start(rep16[:], rep16_p[:])
        slot16 = cpool.tile([16, CFQ], F32, name="slot16")
        nc.sync.dma_start(slot16[:], sl16_p[:])
        if has_rb:
            rbr = cpool.tile([128, E], F32, name="rbr")
            nc.sync.dma_start(rbr[:], rb_p[:])
        if has_b2:
            b2r = cpool.tile([128, D], F32, name="b2r")
            nc.sync.dma_start(b2r[:], b2_p[:])
        if has_sb2:
            sb2r = cpool.tile([128, D], F32, name="sb2r")
            nc.sync.dma_start(sb2r[:], sb2_p[:])
        zt = cpool.tile([128, D], BF16, name="zt")
        nc.vector.memset(zt[:], 0.0)

        # shared-expert rhs for the first half: scalar queue, ahead of
        # everything else scalar so the first shared matmuls start early
        def load_xslbh(half, tag2=""):
            tiles = []
            for d in range(ND):
                t = xslbp.tile(
                    [128, NT // 2], BF16, tag=f"xslbh{d}", name=f"xslbh{half}{tag2}_{d}"
                )
                nc.scalar.dma_start(
                    t[:],
                    xslb_p[d * 128 : (d + 1) * 128, half * 512 : (half + 1) * 512],
                )
                tiles.append(t)
            return tiles

        xslbh0 = load_xslbh(0)

        # ------------- router: rw stationary, tokens streaming ------------
        # gates^T = rw^T @ x : [E, NT] accumulated over d in two 512-chunks
        gps = [
            ps1.tile([E, 512], F32, tag="hpsum", name=f"gps{c}") for c in range(2)
        ]
        for d in range(ND):
            xrd = rp.tile([128, NT], F32, tag="xrd", name=f"xrd{d}", bufs=2)
            nc.sync.dma_start(xrd[:], xsl_p[d * 128 : (d + 1) * 128, :])
            for c in range(2):
                nc.tensor.matmul(
                    gps[c][:],
                    rw_sb[:, d, :],
                    xrd[:, c * 512 : (c + 1) * 512],
                    start=(d == 0),
                    stop=(d == ND - 1),
                )
        gatesT = cpool.tile([E, NT], F32, name="gatesT")
        for c in range(2):
            nc.vector.tensor_copy(gatesT[:, c * 512 : (c + 1) * 512], gps[c][:])
        # transpose to token-major [128, 8j, 8e]
        gallp = ps1.tile([128, NT // 128, E], F32, tag="hpsum", name="gallp")
        for j in range(NT // 128):
            nc.tensor.transpose(
                gallp[:, j, :],
                gatesT[:, j * 128 : (j + 1) * 128],
                ident[0:E, 0:E],
            )
        gall = rp.tile([128, NT // 128, E], F32, tag="gall", name="gall", bufs=1)
        if has_rb:
            nc.vector.tensor_tensor(
                gall[:],
                gallp[:],
                rbr[:].unsqueeze(1).broadcast_to([128, NT // 128, E]),
                op=ALU.add,
            )
        else:
            nc.vector.tensor_copy(gall[:], gallp[:])
        # batched top-2 softmax over the E axis
        JB = NT // 128
        m1 = rp.tile([128, JB], F32, tag="m1", name="m1", bufs=1)
        nc.vector.tensor_reduce(m1[:], gall[:], axis=AX.X, op=ALU.max)
        m1b = m1[:].unsqueeze(2).broadcast_to([128, JB, E])
        mask1 = rp.tile([128, JB, E], F32, tag="mask1", name="mask1", bufs=1)
        nc.vector.tensor_tensor(mask1[:], gall[:], m1b, op=ALU.is_equal)
        gm = rp.tile([128, JB, E], F32, tag="gm", name="gm", bufs=1)
        nc.vector.scalar_tensor_tensor(
            gm[:], mask1[:], -1e30, gall[:], op0=ALU.mult, op1=ALU.add
        )
        m2 = rp.tile([128, JB], F32, tag="m2", name="m2", bufs=1)
        nc.vector.tensor_reduce(m2[:], gm[:], axis=AX.X, op=ALU.max)
        m2b = m2[:].unsqueeze(2).broadcast_to([128, JB, E])
        mask2 = rp.tile([128, JB, E], F32, tag="mask2", name="mask2", bufs=1)
        nc.vector.tensor_tensor(mask2[:], gm[:], m2b, op=ALU.is_equal)
        dl = rp.tile([128, JB], F32, tag="dl", name="dl", bufs=1)
        nc.vector.tensor_tensor(dl[:], m2[:], m1[:], op=ALU.subtract)
        e2 = rp.tile([128, JB], F32, tag="e2", name="e2", bufs=1)
        nc.scalar.activation(e2[:], dl[:], ACTF.Exp)
        den = rp.tile([128, JB], F32, tag="den", name="den", bufs=1)
        nc.vector.tensor_scalar_add(den[:], e2[:], 1.0)
        p1 = rp.tile([128, JB], F32, tag="p1", name="p1", bufs=1)
        nc.vector.reciprocal(p1[:], den[:])
        p2 = rp.tile([128, JB], F32, tag="p2", name="p2", bufs=1)
        nc.vector.tensor_tensor(p2[:], e2[:], p1[:], op=ALU.mult)
        t1 = rp.tile([128, JB, E], F32, tag="t1", name="t1", bufs=1)
        nc.vector.tensor_tensor(
            t1[:], mask1[:], p1[:].unsqueeze(2).broadcast_to([128, JB, E]),
            op=ALU.mult,
        )
        comb = rp.tile([128, JB, E], F32, tag="comb", name="comb", bufs=1)
        nc.vector.tensor_tensor(
            comb[:], mask2[:], p2[:].unsqueeze(2).broadcast_to([128, JB, E]),
            op=ALU.mult,
        )
        nc.vector.tensor_tensor(comb[:], comb[:], t1[:], op=ALU.add)
        # transpose back to [E, NT] for the all-to-all
        combp = ps2.tile([E, NT], F32, tag="ypsum", name="combp")
        for j in range(NT // 128):
            nc.tensor.transpose(
                combp[:, j * 128 : (j + 1) * 128], comb[:, j, :], ident[:]
            )
        # reuse the (now dead) gatesT tile as the A2A staging buffer
        nc.vector.tensor_copy(gatesT[:], combp[:])
        nc.sync.dma_start(a2a_in[:], gatesT[:])
        if debug:
            nc.sync.dma_start(dbg_comb_p[:], comb[:].rearrange("p a b -> p (a b)"))
        nc.gpsimd.collective_compute(
            "AllToAll",
            ALU.bypass,
            replica_groups=RG,
            ins=[a2a_in.opt()],
            outs=[a2a_out.opt()],
        )

        # ------------- shared-expert stage-2 weights -----------------------
        def load_sw2r(s):
            tiles = []
            for h in range(NH):
                t = sw2rp.tile([128, D], BF16, tag=f"sw2r{h}", name=f"sw2r{s}_{h}")
                nc.scalar.dma_start(
                    t[:], sw2_p[s * HE + h * 128 : s * HE + (h + 1) * 128, :]
                )
                tiles.append(t)
            return tiles

        # ------------- shared expert half-pass ----------------------------
        # sw1 streams as per-h column blocks [D, 128] -> [128, ND, 128] on
        # the sync queue (scalar holds the ACTs; interleaving them would
        # serialize the h-pipeline)
        def shared_half(s, half, xslbh, sw2r):
            shts = []
            for h in range(NH):
                swc = sw1cp.tile(
                    [128, ND, 128], BF16, tag="sw1c", name=f"swc{s}_{half}_{h}", bufs=3
                )
                nc.sync.dma_start(
                    swc[:],
                    sw1_p[s * D : (s + 1) * D, h * 128 : (h + 1) * 128].rearrange(
                        "(dd p) f -> p dd f", p=128
                    ),
                )
                ph = ps1.tile([128, NT // 2], F32, tag="hpsum", name=f"shp{s}_{half}_{h}")
                for d in range(ND):
                    nc.tensor.matmul(
                        ph[:],
                        swc[:, d, :],
                        xslbh[d][:],
                        start=(d == 0),
                        stop=(d == ND - 1),
                    )
                sht = hshp.tile(
                    [128, NT // 2], BF16, tag=f"shT{h}", name=f"shT{s}_{half}_{h}"
                )
                nc.scalar.activation(
                    sht[:], ph[:], ACTF.Gelu, bias=sb1v[:, s * NH + h : s * NH + h + 1]
                )
                shts.append(sht)
            for kg in range(2):
                pys = []
                for ki in range(2):
                    pys.append(
                        ps2.tile(
                            [128, D], F32, tag="ypsum", name=f"spy{s}_{half}_{kg}_{ki}"
                        )
                    )
                for h in range(NH):
                    for ki in range(2):
                        for v in range(D // 512):
                            nc.tensor.matmul(
                                pys[ki][:, v * 512 : (v + 1) * 512],
                                shts[h][:, (kg * 2 + ki) * 128 : (kg * 2 + ki + 1) * 128],
                                sw2r[h][:, v * 512 : (v + 1) * 512],
                                start=(h == 0),
                                stop=(h == NH - 1),
                            )
                for ki in range(2):
                    k = half * 4 + kg * 2 + ki
                    yf = yfp.tile([128, D], BF16, tag="yf", name=f"yf{s}_{k}")
                    if s == 0:
                        nc.vector.tensor_copy(yf[:], pys[ki][:])
                    else:
                        shl = ldp.tile([128, D], BF16, tag="ldbf", name=f"shl{s}_{k}")
                        nc.sync.dma_start(shl[:], shared_y[k * 128 : (k + 1) * 128, :])
                        if has_sb2:
                            nc.vector.tensor_tensor(
                                pys[ki][:], pys[ki][:], sb2r[:], op=ALU.add
                            )
                        nc.vector.tensor_tensor(yf[:], pys[ki][:], shl[:], op=ALU.add)
                    nc.sync.dma_start(shared_y[k * 128 : (k + 1) * 128, :], yf[:])

        sw2r0 = load_sw2r(0)
        shared_half(0, 0, xslbh0, sw2r0)

        # routed-expert weights resident; sync transfers land in the window
        # where sync would otherwise idle waiting on the A2A
        w1r = []
        for d in range(ND):
            t = w1rp.tile([128, HE], BF16, tag=f"w1r{d}", name=f"w1r{d}")
            nc.sync.dma_start(t[:], w1_p[d * 128 : (d + 1) * 128, :])
            w1r.append(t)
        w2r = []
        for h in range(NH):
            t = w2rp.tile([128, D], BF16, tag=f"w2r{h}", name=f"w2r{h}")
            nc.sync.dma_start(t[:], w2_p[h * 128 : (h + 1) * 128, :])
            w2r.append(t)

        def zero_fill(q, eng):
            for r2 in range(QTOK // 128):
                eng.dma_start(moe_q[q][r2 * 128 : (r2 + 1) * 128, :], zt[:])

        # ------------- per-quarter compaction + transposed gather ---------
        nf_all = cpool.tile([1, NQ], mybir.dt.uint32, name="nf_all")
        idxL_sb = []
        gate_sb = []
        ix16_sb = []

        def compact(q):
            c16 = cmp_.tile([16, QTOK // 16], F32, tag="c16", name=f"c16_{q}", bufs=2)
            nc.sync.dma_start(
                c16[:].rearrange("p (a f) -> p a f", a=E),
                a2a_out[:, q * 256 : (q + 1) * 256].rearrange(
                    "a (p f) -> p a f", p=16
                ),
            )
            io16q = cmp_.tile([16, QTOK // 16], F32, tag="io16q", name=f"io16q{q}", bufs=2)
            nc.sync.dma_start(io16q[:], io16_p[:, q * 128 : (q + 1) * 128])
            msk = cmp_.tile([16, QTOK // 16], F32, tag="msk", name=f"msk{q}", bufs=2)
            nc.vector.tensor_scalar(msk[:], c16[:], 0.0, None, op0=ALU.not_equal)
            av = cmp_.tile([16, QTOK // 16], F32, tag="av", name=f"av{q}", bufs=2)
            nc.vector.tensor_tensor(av[:], io16q[:], msk[:], op=ALU.mult)
            # msk -> msk-1 in place; av/ag get -1 on unselected entries
            nc.vector.tensor_scalar(msk[:], msk[:], 1.0, None, op0=ALU.subtract)
            nc.vector.tensor_tensor(av[:], av[:], msk[:], op=ALU.add)
            ag = cmp_.tile([16, QTOK // 16], F32, tag="ag", name=f"ag{q}", bufs=2)
            nc.vector.tensor_tensor(ag[:], c16[:], msk[:], op=ALU.add)
            idxc = cmp_.tile([16, CFQ], F32, tag="idxc", name=f"idxc{q}", bufs=2)
            nc.vector.memset(idxc[:], 0.0)
            nfq = cmp_.tile([1, 1], mybir.dt.uint32, tag="nfq", name=f"nfq{q}", bufs=2)
            nc.gpsimd.sparse_gather(idxc[:], av[:], num_found=nfq[:])
            gatec = cmp_.tile([16, CFQ], F32, tag="gatec", name=f"gatec{q}", bufs=2)
            nc.vector.memset(gatec[:], 0.0)
            nfq2 = cmp_.tile([1, 1], mybir.dt.uint32, tag="nfq2", name=f"nfq2{q}", bufs=2)
            nc.gpsimd.sparse_gather(gatec[:], ag[:], num_found=nfq2[:])
            nc.vector.tensor_copy(nf_all[:, q : q + 1], nfq[:])
            # sparse_gather fills the tail beyond num_found with ARBITRARY
            # values -- mask slots >= nf: gate -> 0, idx -> 0 (then += N for
            # the scatter path so pad writes are skipped / hit the pad row)
            nff = cmp_.tile([1, 1], F32, tag="nff", name=f"nff{q}", bufs=2)
            nc.vector.tensor_copy(nff[:], nfq[:])
            nfb = cmp_.tile([128, 1], F32, tag="nfb", name=f"nfb{q}", bufs=2)
            nc.gpsimd.partition_broadcast(nfb[:], nff[:])
            keep = cmp_.tile([16, CFQ], F32, tag="keep", name=f"keep{q}", bufs=2)
            nc.vector.tensor_scalar(
                keep[:], slot16[:], nfb[0:16, :], None, op0=ALU.is_lt
            )
            idxm0 = cmp_.tile([16, CFQ], F32, tag="idxm0", name=f"idxm0{q}", bufs=2)
            nc.vector.tensor_tensor(idxm0[:], idxc[:], keep[:], op=ALU.mult)
            gatecm = cmp_.tile([16, CFQ], F32, tag="gatecm", name=f"gatecm{q}", bufs=2)
            nc.vector.tensor_tensor(gatecm[:], gatec[:], keep[:], op=ALU.mult)
            # gather indices: int16, 16-wrapped, replicated to all 128
            # partitions (each Q7 core reads its own 16-partition window);
            # pad slots point at token 0
            ixr_ps = ps1.tile([128, CFQ], F32, tag="hpsum", name=f"ixr{q}")
            nc.tensor.matmul(ixr_ps[:], rep16[:], idxm0[:], start=True, stop=True)
            ix16 = cmp_.tile(
                [128, CFQ], mybir.dt.int16, tag=f"ix16_{q}", name=f"ix16_{q}"
            )
            nc.vector.tensor_copy(ix16[:], ixr_ps[:])
            ix16_sb.append(ix16)
            # scatter indices: pad slots (gate==0) get idx += N
            gz = cmp_.tile([16, CFQ], F32, tag="gz", name=f"gz{q}", bufs=2)
            nc.vector.tensor_scalar(gz[:], gatecm[:], 0.0, None, op0=ALU.is_equal)
            ixm = cmp_.tile([16, CFQ], F32, tag="ixm", name=f"ixm{q}", bufs=2)
            nc.vector.scalar_tensor_tensor(
                ixm[:], gz[:], float(N), idxm0[:], op0=ALU.mult, op1=ALU.add
            )
            nc.sync.dma_start(
                idx_d[q][:].rearrange("(f p) one -> p (f one)", p=16), ixm[:]
            )
            nc.sync.dma_start(
                gate_d[q][:].rearrange("(f p) one -> p (f one)", p=16), gatecm[:]
            )
            idxf = cmp_.tile([128, CBQ], F32, tag="idxf", name=f"idxf{q}", bufs=2)
            nc.sync.dma_start(
                idxf[:], idx_d[q][:].rearrange("(c p) one -> p (c one)", p=128)
            )
            gs = cmp_.tile([128, CBQ], F32, tag=f"gs{q}", name=f"gs{q}")
            nc.sync.dma_start(
                gs[:], gate_d[q][:].rearrange("(c p) one -> p (c one)", p=128)
            )
            ixl_f = cmp_.tile([128, CBQ], F32, tag="ixlf", name=f"ixlf{q}", bufs=2)
            nc.vector.tensor_scalar(
                ixl_f[:], idxf[:], float(q * QTOK), None, op0=ALU.subtract
            )
            ixl = cmp_.tile([128, CBQ], I32, tag=f"ixl{q}", name=f"ixl{q}")
            nc.vector.tensor_copy(ixl[:], ixl_f[:])
            idxL_sb.append(ixl)
            gate_sb.append(gs)
            if debug:
                nc.sync.dma_start(dbg_gs_p[:, q * CBQ : (q + 1) * CBQ], gs[:])
                nc.sync.dma_start(dbg_ixl_p[:, q * CBQ : (q + 1) * CBQ], ixl_f[:])
                nc.sync.dma_start(dbg_av_p[:, q * 128 : (q + 1) * 128], av[:])
                nc.sync.dma_start(dbg_ag_p[:, q * 128 : (q + 1) * 128], ag[:])
                nc.sync.dma_start(dbg_idxc_p[:, q * CFQ : (q + 1) * CFQ], idxc[:])
                nc.sync.dma_start(dbg_gatec_p[:, q * CFQ : (q + 1) * CFQ], gatec[:])
                nc.sync.dma_start(dbg_ixm_p[:, q * CFQ : (q + 1) * CFQ], ixm[:])
                nc.sync.dma_start(dbg_ix16_p[:, q * CFQ : (q + 1) * CFQ], ix16[:])

        xgt_q = {}

        def emit_gather(q):
            xgt = xgtp.tile([128, ND, CAPQ], BF16, tag="xgt", name=f"xgt{q}")
            nc.gpsimd.dma_gather(
                xgt[:], xtm_p[:], ix16_sb[q][:], CAPQ, CAPQ, D, transpose=True
            )
            xgt_q[q] = xgt

        # ------------- routed quarters: dense tensor stream ---------------
        def quarter_s1(q):
            xgt = xgt_q[q]
            hts = []
            for h in range(NH):
                ph = ps1.tile([128, CAPQ], F32, tag="hpsum", name=f"ph{q}_{h}")
                for d in range(ND):
                    nc.tensor.matmul(
                        ph[:, 0:512],
                        w1r[d][:, h * 128 : (h + 1) * 128],
                        xgt[:, d, 0:512],
                        start=(d == 0),
                        stop=(d == ND - 1),
                    )
                    nc.tensor.matmul(
                        ph[:, 512:CAPQ],
                        w1r[d][:, h * 128 : (h + 1) * 128],
                        xgt[:, d, 512:CAPQ],
                        start=(d == 0),
                        stop=(d == ND - 1),
                    )
                ht = htp.tile([128, CAPQ], BF16, tag=f"hT{h}", name=f"hTe{q}_{h}")
                nc.scalar.activation(ht[:], ph[:], ACTF.Gelu, bias=b1v[:, h : h + 1])
                hts.append(ht)
            if debug and q == 0:
                nc.sync.dma_start(dbg_ht_p[:], hts[0][:])
            return hts

        def quarter_s2(q, hts):
            for c in range(CBQ):
                py = ps2.tile([128, D], F32, tag="ypsum", name=f"py{q}_{c}")
                for h in range(NH):
                    for v in range(D // 512):
                        nc.tensor.matmul(
                            py[:, v * 512 : (v + 1) * 512],
                            hts[h][:, c * 128 : (c + 1) * 128],
                            w2r[h][:, v * 512 : (v + 1) * 512],
                            start=(h == 0),
                            stop=(h == NH - 1),
                        )
                if has_b2:
                    nc.vector.tensor_tensor(py[:], py[:], b2r[:], op=ALU.add)
                ysb = ysbp.tile([128, D], BF16, tag="ysb", name=f"ysbq{q}_{c}")
                nc.vector.tensor_scalar(
                    ysb[:], py[:], gate_sb[q][:, c : c + 1], None, op0=ALU.mult
                )
                if debug and q == 0 and c == 0:
                    nc.sync.dma_start(dbg_ysb_p[:], ysb[:])
                nc.gpsimd.indirect_dma_start(
                    out=moe_q[q][:],
                    out_offset=bass.IndirectOffsetOnAxis(
                        ap=idxL_sb[q][:, c : c + 1], axis=0
                    ),
                    in_=ysb[:],
                    in_offset=None,
                    bounds_check=QTOK + 127,
                    oob_is_err=False,
                )
            if debug and q == 0:
                for r2 in range(QTOK // 128):
                    dt_ = ldp.tile([128, D], BF16, tag="ldbf", name=f"dmoe{r2}")
                    nc.sync.dma_start(dt_[:], moe_q[q][r2 * 128 : (r2 + 1) * 128, :])
                    nc.sync.dma_start(dbg_moe_p[r2 * 128 : (r2 + 1) * 128, :], dt_[:])
            nc.gpsimd.collective_compute(
                "ReduceScatter",
                ALU.add,
                replica_groups=RG,
                ins=[moe_q[q][0:QTOK, :].opt()],
                outs=[rs_q[q].opt()],
            )
            if debug and q == 0:
                for r2 in range(2):
                    dt_ = ldp.tile([128, D], BF16, tag="ldbf", name=f"drs{r2}")
                    nc.sync.dma_start(dt_[:], rs_q[q][r2 * 128 : (r2 + 1) * 128, :])
                    nc.sync.dma_start(dbg_rs_p[r2 * 128 : (r2 + 1) * 128, :], dt_[:])

        def rs_final(q):
            for r in range(2):
                k = q * 2 + r
                rl = ldp.tile([128, D], BF16, tag="ldbf", name=f"rsld{k}")
                nc.sync.dma_start(rl[:], rs_q[q][r * 128 : (r + 1) * 128, :])
                sl = ldp.tile([128, D], BF16, tag="ldbf", name=f"shld{k}")
                nc.sync.dma_start(sl[:], shared_y[k * 128 : (k + 1) * 128, :])
                fo = yfp.tile([128, D], BF16, tag="yf", name=f"fo{k}")
                nc.vector.tensor_tensor(fo[:], rl[:], sl[:], op=ALU.add)
                nc.sync.dma_start(yo_p[k * 128 : (k + 1) * 128, :], fo[:])

        # ------------- orchestration --------------------------------------
        if debug:
            nc.sync.dma_start(gatesT[:], a2a_out[:])
            nc.sync.dma_start(dbg_a2a_p[:], gatesT[:])
        compact(0)
        emit_gather(0)
        if debug:
            nc.sync.dma_start(
                dbg_xgt_p[:], xgt_q[0][:].rearrange("p a b -> p (a b)")
            )
        compact(1)
        emit_gather(1)
        zero_fill(0, nc.sync)
        zero_fill(1, nc.sync)
        xslbh1 = load_xslbh(1)
        shared_half(0, 1, xslbh1, sw2r0)
        zero_fill(2, nc.sync)
        zero_fill(3, nc.sync)
        compact(2)
        compact(3)
        hts0 = quarter_s1(0)
        emit_gather(2)
        quarter_s2(0, hts0)
        xslbh0b = load_xslbh(0, "b")
        sw2r1 = load_sw2r(1)
        shared_half(1, 0, xslbh0b, sw2r1)
        rs_final(0)
        hts1 = quarter_s1(1)
        emit_gather(3)
        quarter_s2(1, hts1)
        hts2 = quarter_s1(2)
        rs_final(1)
        quarter_s2(2, hts2)
        hts3 = quarter_s1(3)
        quarter_s2(3, hts3)
        xslbh1b = load_xslbh(1, "b")
        shared_half(1, 1, xslbh1b, sw2r1)
        rs_final(2)
        rs_final(3)

        nc.sync.dma_start(nf_p[:], nf_all[:])

    nc.compile()
    return nc


def _get_nc(key):
    if key not in _NC_CACHE:
        _NC_CACHE[key] = _build_sparse(*key)
    return _NC_CACHE[key]


def _get_nc_dense(key):
    k2 = ("dense",) + key
    if k2 not in _NC_CACHE:
        _NC_CACHE[k2] = _build(*key)
    return _NC_CACHE[k2]


def _prep_in_maps(x, router_w, router_b, w1, b1, w2, b2, sw1, sb1, sw2, sb2,
                  sparse=True):
    f32 = np.float32
    x2 = np.ascontiguousarray(np.asarray(x, f32).reshape(N, D))
    x2bf = x2.astype(ml_dtypes.bfloat16)
    xt = np.ascontiguousarray(x2.T)

    has_rb = bool(np.any(router_b))
    has_b2 = bool(np.any(b2))
    sb2_eff = np.asarray(sb2, f32).sum(0) / S
    has_sb2 = bool(np.any(sb2_eff))
    key = (has_rb, has_b2, has_sb2)

    rw_r = np.ascontiguousarray(
        np.asarray(router_w, f32).reshape(ND, 128, E).transpose(1, 0, 2)
    )
    ident = np.eye(128, dtype=f32)
    # sparse variant: core e owns 256-token blocks {b : b % 8 == e}.
    # iota16[p, q*128 + a*16 + f] = global id of a2a-flat position
    # u = p*16 + f within (quarter q, core a) = (a + 8q)*256 + p*16 + f
    # (p-major within each 256-token block so the c16 load is contiguous)
    iota16 = np.empty((16, N // 16), f32)
    for q in range(4):
        for a in range(E):
            for f in range(16):
                col = q * 128 + a * 16 + f
                iota16[:, col] = (a + 8 * q) * 256 + np.arange(16) * 16 + f
    rep16 = (np.arange(128)[None, :] % 16 == np.arange(16)[:, None]).astype(f32)
    slot16 = (
        np.arange(CAPQ_HOST // 16)[None, :] * 16 + np.arange(16)[:, None]
    ).astype(f32)
    sw1s = np.ascontiguousarray(np.asarray(sw1, f32).reshape(S * D, HE))
    sw1s_bf = sw1s.astype(ml_dtypes.bfloat16)
    sw2s = np.ascontiguousarray(
        (np.asarray(sw2, f32) * (1.0 / S)).reshape(S * HE, D)
    ).astype(ml_dtypes.bfloat16)
    sb1v = np.ascontiguousarray(
        np.asarray(sb1, f32).reshape(S, NH, 128).transpose(2, 0, 1).reshape(128, S * NH)
    )
    if has_rb:
        rbr = np.tile(np.asarray(router_b, f32), (128, 1))
    if has_sb2:
        sb2r = np.tile(sb2_eff, (128, 1))

    in_maps = []
    for e in range(NCORES):
        if sparse:
            own = np.concatenate(
                [x2[(q * 8 + e) * 256 : (q * 8 + e + 1) * 256] for q in range(4)]
            )
            xsl_e = np.ascontiguousarray(own.T)
        else:
            xsl_e = np.ascontiguousarray(xt[:, e * NT : (e + 1) * NT])
        m = {
            "xsl": xsl_e,
            "w1": np.ascontiguousarray(np.asarray(w1[e], f32)),
            "w2": np.ascontiguousarray(np.asarray(w2[e], f32)).astype(ml_dtypes.bfloat16),
            "rw": rw_r,
            "sw1": sw1s,
            "sw2": sw2s,
            "b1v": np.ascontiguousarray(np.asarray(b1[e], f32).reshape(NH, 128).T),
            "sb1v": sb1v,
            "ident": ident,
        }
        if sparse:
            m["xtm"] = x2bf
            m["xslb"] = xsl_e.astype(ml_dtypes.bfloat16)
            m["iota16"] = iota16
            m["rep16"] = rep16
            m["slot16"] = slot16
            m["w1"] = m["w1"].astype(ml_dtypes.bfloat16)
            m["sw1"] = sw1s_bf
        else:
            m["xt"] = xt
        if has_rb:
            m["rbr"] = rbr
        if has_b2:
            m["b2r"] = np.tile(np.asarray(b2[e], f32), (128, 1))
        if has_sb2:
            m["sb2r"] = sb2r
        in_maps.append(m)
    return key, in_maps


def _install_ntff_hook():
    """Re-create the boot-time NTFF profile hook (this image's antenv lacks
    axon_hooks, so trn_boot degraded silently). Needed only for tracing."""
    import contextlib
    import ctypes
    import types

    try:
        from antenv.axon_hooks import get_axon_ntff_profile_hook  # noqa: F401

        return
    except ImportError:
        pass

    so_path = "/opt/axon/libaxon_pjrt.so"
    lib = ctypes.CDLL(so_path)
    if not hasattr(lib, "axon_start_nrt_profile"):
        return
    lib.axon_start_nrt_profile.argtypes = [
        ctypes.POINTER(ctypes.c_int64),
        ctypes.c_size_t,
    ]
    lib.axon_start_nrt_profile.restype = ctypes.c_int64
    lib.axon_stop_nrt_profile.argtypes = [ctypes.c_char_p]
    lib.axon_stop_nrt_profile.restype = ctypes.c_int64

    @contextlib.contextmanager
    def _hook(output_dir, device_ids):
        import jax

        jax.devices()
        if device_ids:
            ids = (ctypes.c_int64 * len(device_ids))(*device_ids)
            rc = lib.axon_start_nrt_profile(ids, len(device_ids))
        else:
            rc = lib.axon_start_nrt_profile(None, 0)
        if rc != 0:
            raise RuntimeError(f"axon_start_nrt_profile rc={rc}")
        try:
            yield
        finally:
            n = lib.axon_stop_nrt_profile(str(output_dir).encode())
            print(f"profile: {n} file(s) written to {output_dir}", file=sys.stderr)

    mod = types.ModuleType("antenv.axon_hooks")
    mod.get_axon_ntff_profile_hook = lambda: _hook
    mod.set_axon_ntff_profile_hook = lambda h: None
    sys.modules["antenv.axon_hooks"] = mod


def kernel(x, router_w, router_b, w1, b1, w2, b2, sw1, sb1, sw2, sb2, _trace=False):
    if _trace:
        _install_ntff_hook()
    args = (x, router_w, router_b, w1, b1, w2, b2, sw1, sb1, sw2, sb2)
    key, in_maps = _prep_in_maps(*args, sparse=True)
    nc = _get_nc(key)
    res = run_bass_kernel_spmd(
        nc, in_maps, core_ids=list(range(NCORES)), trace=_trace
    )
    counts = [int(c) for e in range(NCORES) for c in res.results[e]["nf_out"]]
    out = np.empty((N, D), np.float32)
    if max(counts) > CAPQ_HOST:
        # capacity overflow (pathologically imbalanced routing):
        # fall back to the dense variant, which is correct for any routing
        key, in_maps = _prep_in_maps(*args, sparse=False)
        nc = _get_nc_dense(key)
        res = run_bass_kernel_spmd(
            nc, in_maps, core_ids=list(range(NCORES)), trace=_trace
        )
        for e in range(NCORES):
            out[e * NT : (e + 1) * NT] = res.results[e]["y_out"]
    else:
        for e in range(NCORES):
            yo = np.asarray(res.results[e]["y_out"], np.float32)
            for q in range(4):
                out[(q * 8 + e) * 256 : (q * 8 + e + 1) * 256] = yo[
                    q * 256 : (q + 1) * 256
                ]
    out = out.reshape(np.asarray(x).shape)
    if _trace:
        return out, res
    return out

